# revision 24
# baseline (speedup 1.0000x reference)
"""Trainium2 Bass kernel for nn_CSFAProV2 — full-input contract.

kernel(**inputs) takes the FULL unsharded inputs (B=32), shards the batch
across 8 NeuronCores (4 samples each, pure data parallel over axis 0 of
x1/x2, weights replicated), compiles+runs the Bass/Tile kernel via
run_bass_kernel_spmd, and concatenates the per-core outputs into the full
[32, 1024, 40, 40] result. Self-contained: the Bass kernel builder is
inlined below; only needs /opt/trn_rl_repo (concourse) + numpy/ml_dtypes.
"""

import sys

if '/opt/trn_rl_repo' not in sys.path:
    sys.path.insert(0, '/opt/trn_rl_repo')

import numpy as np

N_CORES = 8
B_FULL = 32
B_CORE = B_FULL // N_CORES

_cache = {}


def make_in_maps(inputs):
    wd = prep_weights(inputs)
    x1 = np.ascontiguousarray(np.asarray(inputs['x1'], np.float32))
    x2 = np.ascontiguousarray(np.asarray(inputs['x2'], np.float32))

    in_maps = []
    for c in range(N_CORES):
        m = dict(wd)
        m['x1'] = x1[c * B_CORE:(c + 1) * B_CORE]
        m['x2'] = x2[c * B_CORE:(c + 1) * B_CORE]
        in_maps.append(m)
    return in_maps


def kernel(**inputs):
    from concourse.bass_utils import run_bass_kernel_spmd

    if 'nc' not in _cache:
        _cache['nc'] = build_nc(B=B_CORE)
    nc = _cache['nc']

    in_maps = make_in_maps(inputs)
    res = run_bass_kernel_spmd(nc, in_maps, core_ids=list(range(N_CORES)))
    return np.concatenate([res.results[c]['out'] for c in range(N_CORES)], axis=0)


# ======================================================================
# Inlined Bass/Tile kernel builder
# ======================================================================

"""Per-core kernel, B batch samples.

Pipelined structure: stage A(s) = x2 load / patch pool / PE-transposed
value slabs / key proj / scores / softmax partials; stage B(s) = CA gate,
SA convs, attention apply, conv3, conv2. A(s+1) is issued mid-B(s) so its
DMA/vector/scalar work overlaps the conv-heavy PE phase. All matmuls bf16
except where noted; channel-on-partition layouts; convs = PE matmuls
accumulating over (ktile, dy, dx) on padded-spatial SBUF tiles.
"""

import math
import numpy as np
import concourse.bass as bass
import concourse.mybir as mybir
from concourse import bacc
from concourse import masks
from concourse.tile import TileContext
from concourse.alu_op_type import AluOpType

F32 = mybir.dt.float32
F32R = mybir.dt.float32r
BF16 = mybir.dt.bfloat16
AF = mybir.ActivationFunctionType

H = W = 40
WP = 42
PADPIX = WP * WP
CHUNK_ROWS = 10
NCHUNK = H // CHUNK_ROWS
CHUNK_N = CHUNK_ROWS * W


def prep_weights(inp):
    import ml_dtypes
    bf16 = ml_dtypes.bfloat16
    d = {}

    def convT(w, kt_n, mt_n):  # [Cout, Cin, 3, 3] -> [kt, mt, 128, 9*128]
        x = np.asarray(w, np.float32).reshape(mt_n, 128, kt_n, 128, 9)
        x = x.transpose(2, 0, 3, 4, 1)
        return np.ascontiguousarray(x.reshape(kt_n, mt_n, 128, 9 * 128))

    d['qwT'] = convT(inp['q_w'], 4, 2).astype(bf16)
    d['c3wT'] = convT(inp['c3_w'], 4, 4).astype(bf16)
    d['sa1wT'] = convT(inp['sa1_w'], 2, 2).astype(bf16)
    d['sa2wT'] = convT(inp['sa2_w'], 2, 2).astype(bf16)
    w2 = np.asarray(inp['conv2_w'], np.float32)[:, :, 0, 0]
    d['c2wT'] = np.ascontiguousarray(
        w2.reshape(4, 128, 6, 128).transpose(2, 0, 3, 1)).astype(bf16)
    # the patch mean's /16 is folded into the key projection
    kw = np.asarray(inp['key_w'], np.float32) * 0.0625
    d['keywT'] = np.ascontiguousarray(
        kw.reshape(2, 128, 4, 128).transpose(2, 0, 3, 1)).astype(bf16)
    w1 = np.asarray(inp['ca_w1'], np.float32)
    d['caw1T'] = np.ascontiguousarray(
        w1.reshape(64, 2, 128).transpose(1, 2, 0)).astype(bf16)
    w2c = np.asarray(inp['ca_w2'], np.float32)
    d['caw2T'] = np.ascontiguousarray(
        w2c.reshape(2, 128, 64).transpose(2, 0, 1)).astype(bf16)
    for nm, key in [('qs', 'q_s'), ('qb', 'q_b'), ('c3s', 'c3_s'), ('c3b', 'c3_b'),
                    ('sa1s', 'sa1_s'), ('sa1b', 'sa1_b'), ('sa2s', 'sa2_s'),
                    ('sa2b', 'sa2_b'), ('c2s', 'conv2_s'), ('c2b', 'conv2_b'),
                    ('cab1', 'ca_b1'), ('cab2', 'ca_b2')]:
        d[nm] = np.ascontiguousarray(np.asarray(inp[key], np.float32))
    return d


def build_nc(B=4):
    nc = bacc.Bacc(None)
    x1 = nc.dram_tensor("x1", [B, 512, 20, 20], F32, kind="ExternalInput")
    x2 = nc.dram_tensor("x2", [B, 512, 40, 40], F32, kind="ExternalInput")
    w = {}
    w['qwT'] = nc.dram_tensor("qwT", [4, 2, 128, 9 * 128], BF16, kind="ExternalInput")
    w['c3wT'] = nc.dram_tensor("c3wT", [4, 4, 128, 9 * 128], BF16, kind="ExternalInput")
    w['sa1wT'] = nc.dram_tensor("sa1wT", [2, 2, 128, 9 * 128], BF16, kind="ExternalInput")
    w['sa2wT'] = nc.dram_tensor("sa2wT", [2, 2, 128, 9 * 128], BF16, kind="ExternalInput")
    w['c2wT'] = nc.dram_tensor("c2wT", [6, 4, 128, 128], BF16, kind="ExternalInput")
    w['keywT'] = nc.dram_tensor("keywT", [4, 2, 128, 128], BF16, kind="ExternalInput")
    w['caw1T'] = nc.dram_tensor("caw1T", [2, 128, 64], BF16, kind="ExternalInput")
    w['caw2T'] = nc.dram_tensor("caw2T", [64, 2, 128], BF16, kind="ExternalInput")
    for nm, n in [('qs', 256), ('qb', 256), ('c3s', 512), ('c3b', 512),
                  ('sa1s', 256), ('sa1b', 256), ('sa2s', 256), ('sa2b', 256),
                  ('c2s', 512), ('c2b', 512), ('cab1', 64), ('cab2', 256)]:
        w[nm] = nc.dram_tensor(nm, [n], F32, kind="ExternalInput")
    out = nc.dram_tensor("out", [B, 1024, 40, 40], F32, kind="ExternalOutput")

    with TileContext(nc) as tc:
        _emit(nc, tc, B, x1, x2, w, out)
    nc.finalize()
    return nc


def _apron_memset(nc, t):
    nc.gpsimd.memset(t[:, 0:WP], 0.0)
    nc.gpsimd.memset(t[:, 41 * WP:42 * WP], 0.0)
    g = t[:].rearrange("p (y x) -> p y x", x=WP)
    nc.gpsimd.memset(g[:, 1:41, 0:1], 0.0)
    nc.gpsimd.memset(g[:, 1:41, 41:42], 0.0)


def _emit(nc, tc, B, x1, x2, w, out):
    import contextlib
    ctx = contextlib.ExitStack()
    with ctx:
        mp = ctx.enter_context(tc.tile_pool(name="main", bufs=1))
        psC = ctx.enter_context(tc.tile_pool(name="psC", bufs=2, space="PSUM"))
        psT = ctx.enter_context(tc.tile_pool(name="psT", bufs=2, space="PSUM"))
        psY = ctx.enter_context(tc.tile_pool(name="psY", bufs=4, space="PSUM"))

        # ---------- startup: x1 + q-conv weights first ----------
        # x1: load (parallel via xt pool) + zero-padded bf16 [128, (s, 22, 22)]
        x1raw = []
        for ct in range(4):
            raw = mp.tile([128, B * 400], F32, tag="x2", bufs=2)
            src = x1.ap()[:, ct * 128:(ct + 1) * 128].rearrange("s p y x -> p s (y x)")
            nc.sync.dma_start(out=raw[:].rearrange("p (s a) -> p s a", s=B), in_=src)
            x1raw.append(raw)

        qslabs = {}
        for mt in range(2):
            for kt in range(4):
                qbf = mp.tile([128, 1152], BF16, tag="wsbf", bufs=5)
                nc.sync.dma_start(out=qbf[:], in_=w['qwT'][kt, mt])
                qslabs[(kt, mt)] = qbf

        def load_vec(name, n):
            p = min(n, 128)
            t = mp.tile([128, max(n // 128, 1)], F32, tag=f"vec_{name}")
            nc.sync.dma_start(out=t[0:p, 0:max(n // 128, 1)],
                              in_=w[name].ap().rearrange("(a p) -> p a", p=p))
            return t
        vs = {}
        for nm in ('qs', 'qb'):
            vs[nm] = load_vec(nm, 256)

        x1pad = []
        for ct in range(4):
            t = mp.tile([128, B * 484], BF16, tag=f"x1pad{ct}")
            nc.gpsimd.memset(t[:], 0.0)
            dst = t[:].rearrange("p (s y x) -> p s y x", s=B, x=22)[:, :, 1:21, 1:21]
            nc.gpsimd.tensor_copy(
                dst, x1raw[ct][:].rearrange("p (s y x) -> p s y x", s=B, x=20))
            x1pad.append(t)

        ones_bf = mp.tile([128, 1], BF16, tag="ones")
        nc.gpsimd.memset(ones_bf[:], 1.0)
        ident = mp.tile([128, 128], BF16, tag="ident")
        masks.make_identity(nc, ident[:])

        def up_ap(ct, s, chunk):
            y0h = chunk * CHUNK_ROWS // 2
            base = x1pad[ct][:].rearrange("p (ss a) -> p ss a", ss=B)[:, s]
            base = base.rearrange("p (y x) -> p y x", x=22)[:, 1:21, 1:21]
            up = base.unsqueeze(3).broadcast_to([128, 20, 20, 2])
            return up[:, y0h:y0h + 5]

        # ---------- q conv ----------
        q_sb = mp.tile([128, 2 * B * 100], BF16, tag="qsb")
        for mt in range(2):
            ps0 = psC.tile([128, 512], F32, tag="cps", name="cps")
            ps = ps0[:, 0:B * 100]
            first = True
            for kt in range(4):
                base = x1pad[kt][:].rearrange("p (s y x) -> p s y x", s=B, x=22)
                for dy in range(3):
                    for dx in range(3):
                        rhs = base[:, :, dy:dy + 20:2, dx:dx + 20:2]
                        nc.tensor.matmul(
                            ps, qslabs[(kt, mt)][:, (dy * 3 + dx) * 128:(dy * 3 + dx + 1) * 128],
                            rhs, start=first, stop=(kt == 3 and dy == 2 and dx == 2))
                        first = False
            nc.scalar.activation(q_sb[:, mt * B * 100:(mt + 1) * B * 100], ps, AF.Silu,
                                 bias=vs['qb'][:, mt:mt + 1], scale=vs['qs'][:, mt:mt + 1])

        # ---------- remaining resident weights ----------
        keyw_sb = mp.tile([128, 8 * 128], BF16, tag="keyw")
        for kt in range(4):
            nc.sync.dma_start(
                out=keyw_sb[:, kt * 256:(kt + 1) * 256].rearrange("p (m c) -> p m c", m=2),
                in_=w['keywT'][kt].rearrange("m p c -> p m c"))
        caw1_sb = mp.tile([128, 128], BF16, tag="caw1")
        for kt in range(2):
            nc.sync.dma_start(out=caw1_sb[:, kt * 64:(kt + 1) * 64], in_=w['caw1T'][kt])
        caw2_sb = mp.tile([64, 256], BF16, tag="caw2")
        nc.sync.dma_start(out=caw2_sb[:], in_=w['caw2T'].rearrange("p m c -> p (m c)"))
        for nm, n in [('cab1', 64), ('cab2', 256), ('c3s', 512), ('c3b', 512),
                      ('sa1s', 256), ('sa1b', 256), ('sa2s', 256), ('sa2b', 256),
                      ('c2s', 512), ('c2b', 512)]:
            vs[nm] = load_vec(nm, n)

        c2_sb = mp.tile([128, 6 * 512], BF16, tag="c2w")
        for kt in range(6):
            nc.sync.dma_start(
                out=c2_sb[:, kt * 512:(kt + 1) * 512].rearrange("p (m c) -> p m c", m=4),
                in_=w['c2wT'][kt].rearrange("m p c -> p m c"))
        # conv3 weights resident (16 slabs x [128, 1152] bf16); DMAs issued
        # inside the loop at s==0 to keep them off the warmup DMA burst
        c3_sb = mp.tile([128, 16 * 1152], BF16, tag="c3w")

        SCALE = 1.0 / math.sqrt(32)

        # persistent big tensors: aprons zeroed once, interiors rewritten
        attn = []
        for ct in range(4):
            at = mp.tile([128, PADPIX], BF16, tag=f"attn{ct}", name="at", bufs=1)
            attn.append(at)
        xca = []
        xsa1 = []
        a2 = []
        for i in range(2):
            t = mp.tile([128, PADPIX], BF16, tag=f"xca{i}", name="t", bufs=1)
            xca.append(t)
            t2 = mp.tile([128, PADPIX], BF16, tag=f"xsa{i}", name="t2", bufs=1)
            xsa1.append(t2)
            a2t = mp.tile([128, 1600], BF16, tag=f"a2_{i}", name="a2t", bufs=1)
            a2.append(a2t)

        # x2 pp-major bf16 staging slabs (col = pp*128 + k; k in 100:128 zeroed
        # once so DMA transposes of full 128-col blocks move defined data)
        x2bf = []
        for ct in range(4):
            xbf = mp.tile([128, 16 * 128], BF16, tag=f"x2bf{ct}", name="xbf", bufs=1)
            nc.gpsimd.memset(
                xbf[:].rearrange("p (pp c) -> p pp c", pp=16)[:, :, 100:128], 0.0)
            x2bf.append(xbf)

        # ---------- stage A1: x2 load/pool/value-transpose/key-proj ----------
        # pe_transpose: sample 0 builds v_ct on the PE (sync DMA-transpose is
        # too slow to warm up); later samples use DMA transposes on the idle
        # sync engine, issued 1.5 samples ahead of their consumer.
        def stageA1(s, pe_transpose=False):
            xts = []
            for ct in range(4):
                xt = mp.tile([128, 1600], F32, tag="x2", bufs=2)
                nc.sync.dma_start(
                    out=xt[:], in_=x2.ap()[s, ct * 128:(ct + 1) * 128].rearrange("p y x -> p (y x)"))
                xts.append(xt)
            # patch pool (avg folded into key weights)
            kps = []
            for ct in range(4):
                xt = xts[ct]
                p1 = mp.tile([128, 400], F32, tag="pool1", bufs=2)
                vx = xt[:].rearrange("p (y pwx px) -> p y pwx px", pwx=10, px=4)
                pv = p1[:].rearrange("p (y pwx) -> p y pwx", pwx=10)
                nc.vector.tensor_add(pv, vx[:, :, :, 0], vx[:, :, :, 1])
                nc.vector.tensor_add(pv, pv, vx[:, :, :, 2])
                nc.vector.tensor_add(pv, pv, vx[:, :, :, 3])
                vy = p1[:].rearrange("p (phy py pwx) -> p phy py pwx", py=4, pwx=10)
                t01 = mp.tile([128, 100], F32, tag="poolt", bufs=2)
                tv = t01[:].rearrange("p (a b) -> p a b", b=10)
                nc.vector.tensor_add(tv, vy[:, :, 0], vy[:, :, 1])
                nc.vector.tensor_add(tv, tv, vy[:, :, 2])
                kpt = mp.tile([128, 100], BF16, tag="kp", bufs=8)
                nc.vector.tensor_add(kpt[:].rearrange("p (a b) -> p a b", b=10),
                                     tv, vy[:, :, 3])
                kps.append(kpt)
            # regroup to pp-major bf16 (gpsimd), then PE-transpose to v_ct[k, pp*128+f]
            vts = []
            for ct in range(4):
                xt = xts[ct]
                for py in range(4):
                    srcap = xt[:].rearrange("p (phy py pwx px) -> p py phy pwx px",
                                            phy=10, py=4, pwx=10, px=4)[:, py]
                    dstap = x2bf[ct][:].rearrange("p (py px c) -> p py px c", py=4, px=4)[:, py]
                    dstap = dstap[:, :, 0:100].rearrange("p px (phy pwx) -> p phy pwx px", phy=10)
                    nc.gpsimd.tensor_copy(dstap, srcap)
                vt = mp.tile([128, 2048], BF16, tag="v", bufs=8)
                if pe_transpose:
                    for g in range(2):
                        pst = psT.tile([128, 1024], BF16, tag="cpsT", name="cpsT")
                        for j in range(8):
                            pp = g * 8 + j
                            nc.tensor.transpose(pst[0:100, j * 128:(j + 1) * 128],
                                                x2bf[ct][:, pp * 128:pp * 128 + 100], ident[:])
                        nc.vector.tensor_copy(vt[0:100, g * 1024:(g + 1) * 1024], pst[0:100, :])
                else:
                    for pp in range(16):
                        nc.sync.dma_start(out=vt[:, pp * 128:(pp + 1) * 128],
                                          in_=x2bf[ct][:, pp * 128:(pp + 1) * 128],
                                          transpose=True)
                vts.append(vt)
            # key projection
            kk = mp.tile([128, 200], BF16, tag="ksb", bufs=2)
            for mt in range(2):
                psk = psY.tile([128, 512], F32, tag="aps", name="aps")
                for kt in range(4):
                    nc.tensor.matmul(
                        psk[:, 0:100], keyw_sb[:, (kt * 2 + mt) * 128:(kt * 2 + mt + 1) * 128],
                        kps[kt][:], start=(kt == 0), stop=(kt == 3))
                nc.vector.tensor_copy(kk[:, mt * 100:(mt + 1) * 100], psk[:, 0:100])
            return vts, kk

        # ---------- stage A2: scores/softmax + CA gate + gate apply ----------
        def stageA2(s, kk):
            # scores + exp + 1/sum (4 heads share one sum psum tile)
            recs = []
            for quad in range(2):
                pssum = psY.tile([128, 512], F32, tag="aps", name="aps")
                exps = []
                for hh in range(4):
                    h = quad * 4 + hh
                    emb_ct, emb_off = h // 4, (h % 4) * 32
                    pssc = psY.tile([128, 512], F32, tag="aps", name="aps")
                    lhs = kk[emb_off:emb_off + 32, emb_ct * 100:(emb_ct + 1) * 100]
                    rhs = q_sb[emb_off:emb_off + 32, (emb_ct * B + s) * 100:(emb_ct * B + s + 1) * 100]
                    nc.tensor.matmul(pssc[0:100, 0:100], lhs, rhs, tile_position=(emb_off, 0))
                    expT = mp.tile([100, 100], BF16, tag="expT", bufs=12)
                    nc.scalar.activation(expT[:], pssc[0:100, 0:100], AF.Exp, scale=SCALE)
                    nc.tensor.matmul(pssum[0:1, hh * 100:(hh + 1) * 100],
                                     ones_bf[0:100, 0:1], expT[:])
                    exps.append(expT)
                recip4 = mp.tile([1, 400], F32, tag="recip", bufs=2)
                nc.vector.reciprocal(recip4[:], pssum[0:1, 0:400])
                for hh in range(4):
                    rbc = mp.tile([128, 100], F32, tag="rbc", bufs=12)
                    nc.gpsimd.partition_broadcast(rbc[:], recip4[0:1, hh * 100:(hh + 1) * 100])
                    recs.append((exps[hh], rbc))
            # CA gate: all hidden chunks first (grouped RELU), then gates
            hsbs = []
            for chunk in range(NCHUNK):
                # hps layout: col = r*200 + y*40 + x  (pixel (2y+r)*40+x)
                hps = psC.tile([64, 512], F32, tag="cps", name="cps")
                for r in range(2):
                    for i in range(2):
                        nc.tensor.matmul(hps[:, r * 200:(r + 1) * 200],
                                         caw1_sb[:, i * 64:(i + 1) * 64],
                                         up_ap(2 + i, s, chunk),
                                         start=(i == 0 and r == 0),
                                         stop=(i == 1 and r == 1))
                hsb = mp.tile([64, CHUNK_N], BF16, tag="hsb", bufs=5)
                hview = hsb[:].rearrange("p (y r x) -> p y r x", y=5, r=2)
                pview = hps[:, 0:400].rearrange("p (r y x) -> p y r x", r=2, y=5)
                nc.scalar.activation(hview, pview, AF.Relu, bias=vs['cab1'][0:64, 0:1])
                hsbs.append(hsb)
            gates = []
            for chunk in range(NCHUNK):
                gt = mp.tile([128, 2 * CHUNK_N], BF16, tag="gate", bufs=4)
                for mt in range(2):
                    gps = psC.tile([128, 512], F32, tag="cps", name="cps")
                    nc.tensor.matmul(gps[:, 0:400], caw2_sb[0:64, mt * 128:(mt + 1) * 128],
                                     hsbs[chunk][:])
                    nc.scalar.activation(gt[:, mt * CHUNK_N:(mt + 1) * CHUNK_N], gps[:, 0:400],
                                         AF.Sigmoid, bias=vs['cab2'][:, mt:mt + 1])
                gates.append(gt)
            for chunk in range(NCHUNK):
                y0 = chunk * CHUNK_ROWS
                for i in range(2):
                    for r in range(2):
                        dst = xca[i][:].rearrange("p (y x) -> p y x", x=WP)
                        dst = dst[:, 1 + y0 + r:1 + y0 + 10:2, 1:41]
                        g = gates[chunk][:, i * CHUNK_N:(i + 1) * CHUNK_N]
                        g = g.rearrange("p (y x) -> p y x", x=40)[:, r::2]
                        nc.vector.tensor_tensor(dst, up_ap(2 + i, s, chunk), g, AluOpType.mult)
            return recs

        def stream_sa(wt):
            slabs = {}
            for mt in range(2):
                for kt in range(2):
                    tl = mp.tile([128, 1152], BF16, tag="wsbf", bufs=5)
                    nc.sync.dma_start(out=tl[:], in_=wt[kt, mt])
                    slabs[(kt, mt)] = tl
            return slabs

        def conv3x3(src_tiles, slab_fn, mt, kt_n, chunk):
            ps0 = psC.tile([128, 512], F32, tag="cps", name="cps")
            ps = ps0[:, 0:CHUNK_N]
            y0 = chunk * CHUNK_ROWS
            first = True
            for kt in range(kt_n):
                base = src_tiles[kt][:].rearrange("p (y x) -> p y x", x=WP)
                for dy in range(3):
                    for dx in range(3):
                        rhs = base[:, y0 + dy:y0 + dy + CHUNK_ROWS, dx:dx + 40]
                        lhsT = slab_fn(kt, mt, dy * 3 + dx)
                        nc.tensor.matmul(ps, lhsT, rhs, start=first,
                                         stop=(kt == kt_n - 1 and dy == 2 and dx == 2))
                        first = False
            return ps

        # ---------- main loop ----------
        vts, kk0 = stageA1(0, pe_transpose=True)
        for t in attn + xca + xsa1:
            _apron_memset(nc, t)
        recs = stageA2(0, kk0)
        for s in range(B):
            # ---- SA conv1 ----
            sa1slabs = stream_sa(w['sa1wT'])
            for mt in range(2):
                for chunk in range(NCHUNK):
                    ps = conv3x3(xca, lambda kt, m, t: sa1slabs[(kt, m)][:, t * 128:(t + 1) * 128],
                                 mt, 2, chunk)[:, 0:CHUNK_N]
                    y0 = chunk * CHUNK_ROWS
                    dst = xsa1[mt][:].rearrange("p (y x) -> p y x", x=WP)[:, 1 + y0:11 + y0, 1:41]
                    nc.scalar.activation(dst, ps.rearrange("p (a b) -> p a b", b=40), AF.Silu,
                                         bias=vs['sa1b'][:, mt:mt + 1], scale=vs['sa1s'][:, mt:mt + 1])

            # ---- attention apply: 4 patch-positions per psum bank ----
            for h in range(8):
                expT, rbc = recs[h]
                o = (h % 2) * 64
                for py in range(4):
                    psy = psY.tile([128, 512], F32, tag="aps", name="aps")
                    for px in range(4):
                        pp = py * 4 + px
                        lhsT = vts[h // 2][0:100, pp * 128 + o: pp * 128 + o + 64]
                        nc.tensor.matmul(psy[o:o + 64, px * 100:px * 100 + 100], lhsT, expT[:],
                                         start=(px == 0), stop=(px == 3))
                    dstg = attn[h // 2][o:o + 64, :].rearrange("p (y x) -> p y x", x=WP)
                    dstg = dstg[:, 1 + py:38 + py:4, 1:41]
                    dstg = dstg.rearrange("p a (pwx px) -> p a pwx px", px=4)
                    in0 = psy[o:o + 64, 0:400].rearrange("p (px phy pwx) -> p phy pwx px",
                                                         px=4, phy=10)
                    in1 = rbc[o:o + 64, :].rearrange("p (a b) -> p a b", b=10)
                    in1 = in1.unsqueeze(3).broadcast_to([64, 10, 10, 4])
                    nc.vector.scalar_tensor_tensor(dstg, in0, 0.0, in1,
                                                   AluOpType.bypass, AluOpType.mult)

            # ---- prefetch next sample's A1 stage ----
            if s + 1 < B:
                next_vts, next_kk = stageA1(s + 1)

            # ---- SA conv2 + residual ----
            sa2slabs = stream_sa(w['sa2wT'])
            for mt in range(2):
                for chunk in range(NCHUNK):
                    ps = conv3x3(xsa1, lambda kt, m, t: sa2slabs[(kt, m)][:, t * 128:(t + 1) * 128],
                                 mt, 2, chunk)[:, 0:CHUNK_N]
                    y0 = chunk * CHUNK_ROWS
                    tsilu = mp.tile([128, CHUNK_N], F32, tag="silu", bufs=3)
                    nc.scalar.activation(tsilu[:], ps, AF.Silu,
                                         bias=vs['sa2b'][:, mt:mt + 1], scale=vs['sa2s'][:, mt:mt + 1])
                    xc = xca[mt][:].rearrange("p (y x) -> p y x", x=WP)[:, 1 + y0:11 + y0, 1:41]
                    nc.vector.tensor_tensor(a2[mt][:, y0 * 40:(y0 + 10) * 40],
                                            tsilu[:].rearrange("p (a b) -> p a b", b=40),
                                            xc, AluOpType.add)

            # ---- conv3 (resident bf16 slabs) + residual -> x2_out ----
            if s == 0:
                for mt in range(4):
                    for kt in range(4):
                        off = (mt * 4 + kt) * 1152
                        nc.sync.dma_start(out=c3_sb[:, off:off + 1152], in_=w['c3wT'][kt, mt])
            for mt in range(4):
                for chunk in range(NCHUNK):
                    ps = conv3x3(
                        attn,
                        lambda kt, m, t: c3_sb[:, (m * 4 + kt) * 1152 + t * 128:
                                               (m * 4 + kt) * 1152 + (t + 1) * 128],
                        mt, 4, chunk)[:, 0:CHUNK_N]
                    y0 = chunk * CHUNK_ROWS
                    tsilu = mp.tile([128, CHUNK_N], F32, tag="silu", bufs=3)
                    nc.scalar.activation(tsilu[:], ps, AF.Silu,
                                         bias=vs['c3b'][:, mt:mt + 1], scale=vs['c3s'][:, mt:mt + 1])
                    osb = mp.tile([128, CHUNK_N], F32, tag="osb", bufs=3)
                    at2 = attn[mt][:].rearrange("p (y x) -> p y x", x=WP)[:, 1 + y0:11 + y0, 1:41]
                    nc.vector.tensor_tensor(osb[:].rearrange("p (a b) -> p a b", b=40),
                                            tsilu[:].rearrange("p (a b) -> p a b", b=40),
                                            at2, AluOpType.add)
                    nc.sync.dma_start(
                        out=out.ap()[s, 512 + mt * 128:512 + (mt + 1) * 128]
                            .rearrange("p y x -> p (y x)")[:, y0 * 40:(y0 + 10) * 40],
                        in_=osb[:])

            # ---- prefetch next sample's A2 stage (scores/softmax + CA gate) ----
            if s + 1 < B:
                next_recs = stageA2(s + 1, next_kk)

            # ---- conv2 (1x1) -> x1_out ----
            for mt in range(4):
                for chunk in range(NCHUNK):
                    # ps layout: col = r*200 + y*40 + x  (pixel (2y+r)*40+x)
                    ps0 = psC.tile([128, 512], F32, tag="cps", name="cps")
                    ps = ps0[:, 0:CHUNK_N]
                    y0 = chunk * CHUNK_ROWS
                    first = True
                    for r in range(2):
                        for kt in range(4):
                            nc.tensor.matmul(
                                ps[:, r * 200:(r + 1) * 200],
                                c2_sb[:, (kt * 4 + mt) * 128:(kt * 4 + mt + 1) * 128],
                                up_ap(kt, s, chunk), start=first, stop=False)
                            first = False
                    for i in range(2):
                        kt = 4 + i
                        rhs = a2[i][:, y0 * 40:(y0 + 10) * 40]
                        rhs = rhs.rearrange("p (y r x) -> p r y x", y=5, r=2)
                        nc.tensor.matmul(ps, c2_sb[:, (kt * 4 + mt) * 128:(kt * 4 + mt + 1) * 128],
                                         rhs, start=False, stop=(i == 1))
                    osb = mp.tile([128, CHUNK_N], F32, tag="osb", bufs=3)
                    oview = osb[:].rearrange("p (y r x) -> p y r x", y=5, r=2)
                    pv = ps.rearrange("p (r y x) -> p y r x", r=2, y=5)
                    nc.scalar.activation(oview, pv, AF.Silu,
                                         bias=vs['c2b'][:, mt:mt + 1], scale=vs['c2s'][:, mt:mt + 1])
                    nc.sync.dma_start(
                        out=out.ap()[s, mt * 128:(mt + 1) * 128]
                            .rearrange("p y x -> p (y x)")[:, y0 * 40:(y0 + 10) * 40],
                        in_=osb[:])

            if s + 1 < B:
                vts, recs = next_vts, next_recs


# revision 26
# speedup vs baseline: 1.3470x; 1.3470x over previous
"""Trainium2 Bass kernel for nn_CSFAProV2 — full-input contract.

kernel(**inputs) takes the FULL unsharded inputs (B=32), shards the batch
across 8 NeuronCores (4 samples each, pure data parallel over axis 0 of
x1/x2, weights replicated), compiles+runs the Bass/Tile kernel via
run_bass_kernel_spmd, and concatenates the per-core outputs into the full
[32, 1024, 40, 40] result. Self-contained: the Bass kernel builder is
inlined below; only needs /opt/trn_rl_repo (concourse) + numpy/ml_dtypes.
"""

import sys

if '/opt/trn_rl_repo' not in sys.path:
    sys.path.insert(0, '/opt/trn_rl_repo')

import numpy as np

N_CORES = 8
B_FULL = 32
B_CORE = B_FULL // N_CORES

_cache = {}


def make_in_maps(inputs):
    wd = prep_weights(inputs)
    x1 = np.ascontiguousarray(np.asarray(inputs['x1'], np.float32))
    x2 = np.ascontiguousarray(np.asarray(inputs['x2'], np.float32))

    in_maps = []
    for c in range(N_CORES):
        m = dict(wd)
        m['x1'] = x1[c * B_CORE:(c + 1) * B_CORE]
        m['x2'] = x2[c * B_CORE:(c + 1) * B_CORE]
        in_maps.append(m)
    return in_maps


def kernel(**inputs):
    from concourse.bass_utils import run_bass_kernel_spmd

    if 'nc' not in _cache:
        _cache['nc'] = build_nc(B=B_CORE)
    nc = _cache['nc']

    in_maps = make_in_maps(inputs)
    res = run_bass_kernel_spmd(nc, in_maps, core_ids=list(range(N_CORES)))
    return np.concatenate([res.results[c]['out'] for c in range(N_CORES)], axis=0)


# ======================================================================
# Inlined Bass/Tile kernel builder
# ======================================================================

"""Per-core kernel, B batch samples.

Pipelined structure: stage A(s) = x2 load / patch pool / PE-transposed
value slabs / key proj / scores / softmax partials; stage B(s) = CA gate,
SA convs, attention apply, conv3, conv2. A(s+1) is issued mid-B(s) so its
DMA/vector/scalar work overlaps the conv-heavy PE phase. All matmuls bf16
except where noted; channel-on-partition layouts; convs = PE matmuls
accumulating over (ktile, dy, dx) on padded-spatial SBUF tiles.
"""

import math
import numpy as np
import concourse.bass as bass
import concourse.mybir as mybir
from concourse import bacc
from concourse import masks
from concourse.tile import TileContext
from concourse.alu_op_type import AluOpType

F32 = mybir.dt.float32
F32R = mybir.dt.float32r
BF16 = mybir.dt.bfloat16
AF = mybir.ActivationFunctionType

H = W = 40
WP = 42
PADPIX = WP * WP
CHUNK_ROWS = 10
NCHUNK = H // CHUNK_ROWS
CHUNK_N = CHUNK_ROWS * W


def prep_weights(inp):
    import ml_dtypes
    bf16 = ml_dtypes.bfloat16
    d = {}

    def convT(w, kt_n, mt_n):  # [Cout, Cin, 3, 3] -> [kt, mt, 128, 9*128]
        x = np.asarray(w, np.float32).reshape(mt_n, 128, kt_n, 128, 9)
        x = x.transpose(2, 0, 3, 4, 1)
        return np.ascontiguousarray(x.reshape(kt_n, mt_n, 128, 9 * 128))

    d['qwT'] = convT(inp['q_w'], 4, 2).astype(bf16)
    d['c3wT'] = convT(inp['c3_w'], 4, 4).astype(bf16)
    d['sa1wT'] = convT(inp['sa1_w'], 2, 2).astype(bf16)
    d['sa2wT'] = convT(inp['sa2_w'], 2, 2).astype(bf16)
    w2 = np.asarray(inp['conv2_w'], np.float32)[:, :, 0, 0]
    d['c2wT'] = np.ascontiguousarray(
        w2.reshape(4, 128, 6, 128).transpose(2, 0, 3, 1)).astype(bf16)
    # the patch mean's /16 is folded into the key projection
    kw = np.asarray(inp['key_w'], np.float32) * 0.0625
    d['keywT'] = np.ascontiguousarray(
        kw.reshape(2, 128, 4, 128).transpose(2, 0, 3, 1)).astype(bf16)
    w1 = np.asarray(inp['ca_w1'], np.float32)
    d['caw1T'] = np.ascontiguousarray(
        w1.reshape(64, 2, 128).transpose(1, 2, 0)).astype(bf16)
    w2c = np.asarray(inp['ca_w2'], np.float32)
    d['caw2T'] = np.ascontiguousarray(
        w2c.reshape(2, 128, 64).transpose(2, 0, 1)).astype(bf16)
    for nm, key in [('qs', 'q_s'), ('qb', 'q_b'), ('c3s', 'c3_s'), ('c3b', 'c3_b'),
                    ('sa1s', 'sa1_s'), ('sa1b', 'sa1_b'), ('sa2s', 'sa2_s'),
                    ('sa2b', 'sa2_b'), ('c2s', 'conv2_s'), ('c2b', 'conv2_b'),
                    ('cab1', 'ca_b1'), ('cab2', 'ca_b2')]:
        d[nm] = np.ascontiguousarray(np.asarray(inp[key], np.float32))
    return d


def build_nc(B=4):
    nc = bacc.Bacc(None)
    x1 = nc.dram_tensor("x1", [B, 512, 20, 20], F32, kind="ExternalInput")
    x2 = nc.dram_tensor("x2", [B, 512, 40, 40], F32, kind="ExternalInput")
    w = {}
    w['qwT'] = nc.dram_tensor("qwT", [4, 2, 128, 9 * 128], BF16, kind="ExternalInput")
    w['c3wT'] = nc.dram_tensor("c3wT", [4, 4, 128, 9 * 128], BF16, kind="ExternalInput")
    w['sa1wT'] = nc.dram_tensor("sa1wT", [2, 2, 128, 9 * 128], BF16, kind="ExternalInput")
    w['sa2wT'] = nc.dram_tensor("sa2wT", [2, 2, 128, 9 * 128], BF16, kind="ExternalInput")
    w['c2wT'] = nc.dram_tensor("c2wT", [6, 4, 128, 128], BF16, kind="ExternalInput")
    w['keywT'] = nc.dram_tensor("keywT", [4, 2, 128, 128], BF16, kind="ExternalInput")
    w['caw1T'] = nc.dram_tensor("caw1T", [2, 128, 64], BF16, kind="ExternalInput")
    w['caw2T'] = nc.dram_tensor("caw2T", [64, 2, 128], BF16, kind="ExternalInput")
    for nm, n in [('qs', 256), ('qb', 256), ('c3s', 512), ('c3b', 512),
                  ('sa1s', 256), ('sa1b', 256), ('sa2s', 256), ('sa2b', 256),
                  ('c2s', 512), ('c2b', 512), ('cab1', 64), ('cab2', 256)]:
        w[nm] = nc.dram_tensor(nm, [n], F32, kind="ExternalInput")
    out = nc.dram_tensor("out", [B, 1024, 40, 40], F32, kind="ExternalOutput")

    with TileContext(nc) as tc:
        _emit(nc, tc, B, x1, x2, w, out)
    nc.finalize()
    return nc


def _apron_memset(nc, t):
    nc.gpsimd.memset(t[:, 0:WP], 0.0)
    nc.gpsimd.memset(t[:, 41 * WP:42 * WP], 0.0)
    g = t[:].rearrange("p (y x) -> p y x", x=WP)
    nc.gpsimd.memset(g[:, 1:41, 0:1], 0.0)
    nc.gpsimd.memset(g[:, 1:41, 41:42], 0.0)


def _emit(nc, tc, B, x1, x2, w, out):
    import contextlib
    ctx = contextlib.ExitStack()
    with ctx:
        mp = ctx.enter_context(tc.tile_pool(name="main", bufs=1))
        psC = ctx.enter_context(tc.tile_pool(name="psC", bufs=2, space="PSUM"))
        psT = ctx.enter_context(tc.tile_pool(name="psT", bufs=2, space="PSUM"))
        psY = ctx.enter_context(tc.tile_pool(name="psY", bufs=4, space="PSUM"))

        # ---------- startup: x1 + q-conv weights first ----------
        # x1: load (parallel via xt pool) + zero-padded bf16 [128, (s, 22, 22)]
        x1raw = []
        for ct in range(4):
            raw = mp.tile([128, B * 400], F32, tag="x2", bufs=2)
            src = x1.ap()[:, ct * 128:(ct + 1) * 128].rearrange("s p y x -> p s (y x)")
            nc.sync.dma_start(out=raw[:].rearrange("p (s a) -> p s a", s=B), in_=src)
            x1raw.append(raw)

        qslabs = {}
        for mt in range(2):
            for kt in range(4):
                qbf = mp.tile([128, 1152], BF16, tag="wsbf", bufs=5)
                nc.sync.dma_start(out=qbf[:], in_=w['qwT'][kt, mt])
                qslabs[(kt, mt)] = qbf

        def load_vec(name, n):
            p = min(n, 128)
            t = mp.tile([128, max(n // 128, 1)], F32, tag=f"vec_{name}")
            nc.sync.dma_start(out=t[0:p, 0:max(n // 128, 1)],
                              in_=w[name].ap().rearrange("(a p) -> p a", p=p))
            return t
        vs = {}
        for nm in ('qs', 'qb'):
            vs[nm] = load_vec(nm, 256)

        x1pad = []
        for ct in range(4):
            t = mp.tile([128, B * 484], BF16, tag=f"x1pad{ct}")
            nc.gpsimd.memset(t[:], 0.0)
            dst = t[:].rearrange("p (s y x) -> p s y x", s=B, x=22)[:, :, 1:21, 1:21]
            nc.gpsimd.tensor_copy(
                dst, x1raw[ct][:].rearrange("p (s y x) -> p s y x", s=B, x=20))
            x1pad.append(t)

        ones_bf = mp.tile([128, 1], BF16, tag="ones")
        nc.gpsimd.memset(ones_bf[:], 1.0)
        ident = mp.tile([128, 128], BF16, tag="ident")
        masks.make_identity(nc, ident[:])

        def up_ap(ct, s, chunk):
            y0h = chunk * CHUNK_ROWS // 2
            base = x1pad[ct][:].rearrange("p (ss a) -> p ss a", ss=B)[:, s]
            base = base.rearrange("p (y x) -> p y x", x=22)[:, 1:21, 1:21]
            up = base.unsqueeze(3).broadcast_to([128, 20, 20, 2])
            return up[:, y0h:y0h + 5]

        # ---------- q conv ----------
        q_sb = mp.tile([128, 2 * B * 100], BF16, tag="qsb")
        for mt in range(2):
            ps0 = psC.tile([128, 512], F32, tag="cps", name="cps")
            ps = ps0[:, 0:B * 100]
            first = True
            for kt in range(4):
                base = x1pad[kt][:].rearrange("p (s y x) -> p s y x", s=B, x=22)
                for dy in range(3):
                    for dx in range(3):
                        rhs = base[:, :, dy:dy + 20:2, dx:dx + 20:2]
                        nc.tensor.matmul(
                            ps, qslabs[(kt, mt)][:, (dy * 3 + dx) * 128:(dy * 3 + dx + 1) * 128],
                            rhs, start=first, stop=(kt == 3 and dy == 2 and dx == 2))
                        first = False
            nc.scalar.activation(q_sb[:, mt * B * 100:(mt + 1) * B * 100], ps, AF.Silu,
                                 bias=vs['qb'][:, mt:mt + 1], scale=vs['qs'][:, mt:mt + 1])

        # ---------- remaining resident weights ----------
        keyw_sb = mp.tile([128, 8 * 128], BF16, tag="keyw")
        for kt in range(4):
            nc.sync.dma_start(
                out=keyw_sb[:, kt * 256:(kt + 1) * 256].rearrange("p (m c) -> p m c", m=2),
                in_=w['keywT'][kt].rearrange("m p c -> p m c"))
        caw1_sb = mp.tile([128, 128], BF16, tag="caw1")
        for kt in range(2):
            nc.sync.dma_start(out=caw1_sb[:, kt * 64:(kt + 1) * 64], in_=w['caw1T'][kt])
        caw2_sb = mp.tile([64, 256], BF16, tag="caw2")
        nc.sync.dma_start(out=caw2_sb[:], in_=w['caw2T'].rearrange("p m c -> p (m c)"))
        for nm, n in [('cab1', 64), ('cab2', 256), ('c3s', 512), ('c3b', 512),
                      ('sa1s', 256), ('sa1b', 256), ('sa2s', 256), ('sa2b', 256),
                      ('c2s', 512), ('c2b', 512)]:
            vs[nm] = load_vec(nm, n)

        c2_sb = mp.tile([128, 6 * 512], BF16, tag="c2w")
        for kt in range(6):
            nc.sync.dma_start(
                out=c2_sb[:, kt * 512:(kt + 1) * 512].rearrange("p (m c) -> p m c", m=4),
                in_=w['c2wT'][kt].rearrange("m p c -> p m c"))
        # conv3 weights resident (16 slabs x [128, 1152] bf16); DMAs issued
        # inside the loop at s==0 to keep them off the warmup DMA burst
        c3_sb = mp.tile([128, 16 * 1152], BF16, tag="c3w")

        SCALE = 1.0 / math.sqrt(32)

        # persistent big tensors: aprons zeroed once, interiors rewritten
        attn = []
        for ct in range(4):
            at = mp.tile([128, PADPIX], BF16, tag=f"attn{ct}", name="at", bufs=1)
            attn.append(at)
        xca = []
        xsa1 = []
        a2 = []
        for i in range(2):
            t = mp.tile([128, PADPIX], BF16, tag=f"xca{i}", name="t", bufs=1)
            xca.append(t)
            t2 = mp.tile([128, PADPIX], BF16, tag=f"xsa{i}", name="t2", bufs=1)
            xsa1.append(t2)
            a2t = mp.tile([128, 1600], BF16, tag=f"a2_{i}", name="a2t", bufs=1)
            a2.append(a2t)

        # x2 pp-major bf16 staging slabs (col = pp*128 + k; k in 100:128 zeroed
        # once so DMA transposes of full 128-col blocks move defined data)
        x2bf = []
        for ct in range(4):
            xbf = mp.tile([128, 16 * 128], BF16, tag=f"x2bf{ct}", name="xbf", bufs=1)
            nc.gpsimd.memset(
                xbf[:].rearrange("p (pp c) -> p pp c", pp=16)[:, :, 100:128], 0.0)
            x2bf.append(xbf)

        # ---------- stage A1: x2 load/pool/value-transpose/key-proj ----------
        # pe_transpose: sample 0 builds v_ct on the PE (sync DMA-transpose is
        # too slow to warm up); later samples use DMA transposes on the idle
        # sync engine, issued 1.5 samples ahead of their consumer.
        def stageA1(s, pe_transpose=True):
            xts = []
            for ct in range(4):
                xt = mp.tile([128, 1600], F32, tag="x2", bufs=2)
                nc.sync.dma_start(
                    out=xt[:], in_=x2.ap()[s, ct * 128:(ct + 1) * 128].rearrange("p y x -> p (y x)"))
                xts.append(xt)
            # patch pool (avg folded into key weights)
            kps = []
            for ct in range(4):
                xt = xts[ct]
                p1 = mp.tile([128, 400], F32, tag="pool1", bufs=2)
                vx = xt[:].rearrange("p (y pwx px) -> p y pwx px", pwx=10, px=4)
                pv = p1[:].rearrange("p (y pwx) -> p y pwx", pwx=10)
                nc.vector.tensor_add(pv, vx[:, :, :, 0], vx[:, :, :, 1])
                nc.vector.tensor_add(pv, pv, vx[:, :, :, 2])
                nc.vector.tensor_add(pv, pv, vx[:, :, :, 3])
                vy = p1[:].rearrange("p (phy py pwx) -> p phy py pwx", py=4, pwx=10)
                t01 = mp.tile([128, 100], F32, tag="poolt", bufs=2)
                tv = t01[:].rearrange("p (a b) -> p a b", b=10)
                nc.vector.tensor_add(tv, vy[:, :, 0], vy[:, :, 1])
                nc.vector.tensor_add(tv, tv, vy[:, :, 2])
                kpt = mp.tile([128, 100], BF16, tag="kp", bufs=8)
                nc.vector.tensor_add(kpt[:].rearrange("p (a b) -> p a b", b=10),
                                     tv, vy[:, :, 3])
                kps.append(kpt)
            # regroup to pp-major bf16 (gpsimd), then PE-transpose to v_ct[k, pp*128+f]
            vts = []
            for ct in range(4):
                xt = xts[ct]
                for py in range(4):
                    srcap = xt[:].rearrange("p (phy py pwx px) -> p py phy pwx px",
                                            phy=10, py=4, pwx=10, px=4)[:, py]
                    dstap = x2bf[ct][:].rearrange("p (py px c) -> p py px c", py=4, px=4)[:, py]
                    dstap = dstap[:, :, 0:100].rearrange("p px (phy pwx) -> p phy pwx px", phy=10)
                    nc.gpsimd.tensor_copy(dstap, srcap)
                vt = mp.tile([128, 2048], BF16, tag="v", bufs=8)
                if pe_transpose:
                    for g in range(2):
                        pst = psT.tile([128, 1024], BF16, tag="cpsT", name="cpsT")
                        for j in range(8):
                            pp = g * 8 + j
                            nc.tensor.transpose(pst[0:100, j * 128:(j + 1) * 128],
                                                x2bf[ct][:, pp * 128:pp * 128 + 100], ident[:])
                        nc.vector.tensor_copy(vt[0:100, g * 1024:(g + 1) * 1024], pst[0:100, :])
                else:
                    for pp in range(16):
                        nc.sync.dma_start(out=vt[:, pp * 128:(pp + 1) * 128],
                                          in_=x2bf[ct][:, pp * 128:(pp + 1) * 128],
                                          transpose=True)
                vts.append(vt)
            # key projection
            kk = mp.tile([128, 200], BF16, tag="ksb", bufs=2)
            for mt in range(2):
                psk = psY.tile([128, 512], F32, tag="aps", name="aps")
                for kt in range(4):
                    nc.tensor.matmul(
                        psk[:, 0:100], keyw_sb[:, (kt * 2 + mt) * 128:(kt * 2 + mt + 1) * 128],
                        kps[kt][:], start=(kt == 0), stop=(kt == 3))
                nc.vector.tensor_copy(kk[:, mt * 100:(mt + 1) * 100], psk[:, 0:100])
            return vts, kk

        # ---------- stage A2: scores/softmax + CA gate + gate apply ----------
        def stageA2(s, kk):
            # scores + exp + 1/sum (4 heads share one sum psum tile)
            recs = []
            for quad in range(2):
                pssum = psY.tile([128, 512], F32, tag="aps", name="aps")
                exps = []
                for hh in range(4):
                    h = quad * 4 + hh
                    emb_ct, emb_off = h // 4, (h % 4) * 32
                    pssc = psY.tile([128, 512], F32, tag="aps", name="aps")
                    lhs = kk[emb_off:emb_off + 32, emb_ct * 100:(emb_ct + 1) * 100]
                    rhs = q_sb[emb_off:emb_off + 32, (emb_ct * B + s) * 100:(emb_ct * B + s + 1) * 100]
                    nc.tensor.matmul(pssc[0:100, 0:100], lhs, rhs, tile_position=(emb_off, 0))
                    expT = mp.tile([100, 100], BF16, tag="expT", bufs=12)
                    nc.scalar.activation(expT[:], pssc[0:100, 0:100], AF.Exp, scale=SCALE)
                    nc.tensor.matmul(pssum[0:1, hh * 100:(hh + 1) * 100],
                                     ones_bf[0:100, 0:1], expT[:])
                    exps.append(expT)
                recip4 = mp.tile([1, 400], F32, tag="recip", bufs=2)
                nc.vector.reciprocal(recip4[:], pssum[0:1, 0:400])
                for hh in range(4):
                    rbc = mp.tile([128, 100], F32, tag="rbc", bufs=12)
                    nc.gpsimd.partition_broadcast(rbc[:], recip4[0:1, hh * 100:(hh + 1) * 100])
                    recs.append((exps[hh], rbc))
            # CA gate: all hidden chunks first (grouped RELU), then gates
            hsbs = []
            for chunk in range(NCHUNK):
                # hps layout: col = r*200 + y*40 + x  (pixel (2y+r)*40+x)
                hps = psC.tile([64, 512], F32, tag="cps", name="cps")
                for r in range(2):
                    for i in range(2):
                        nc.tensor.matmul(hps[:, r * 200:(r + 1) * 200],
                                         caw1_sb[:, i * 64:(i + 1) * 64],
                                         up_ap(2 + i, s, chunk),
                                         start=(i == 0 and r == 0),
                                         stop=(i == 1 and r == 1))
                hsb = mp.tile([64, CHUNK_N], BF16, tag="hsb", bufs=5)
                hview = hsb[:].rearrange("p (y r x) -> p y r x", y=5, r=2)
                pview = hps[:, 0:400].rearrange("p (r y x) -> p y r x", r=2, y=5)
                nc.scalar.activation(hview, pview, AF.Relu, bias=vs['cab1'][0:64, 0:1])
                hsbs.append(hsb)
            gates = []
            for chunk in range(NCHUNK):
                gt = mp.tile([128, 2 * CHUNK_N], BF16, tag="gate", bufs=4)
                for mt in range(2):
                    gps = psC.tile([128, 512], F32, tag="cps", name="cps")
                    nc.tensor.matmul(gps[:, 0:400], caw2_sb[0:64, mt * 128:(mt + 1) * 128],
                                     hsbs[chunk][:])
                    nc.scalar.activation(gt[:, mt * CHUNK_N:(mt + 1) * CHUNK_N], gps[:, 0:400],
                                         AF.Sigmoid, bias=vs['cab2'][:, mt:mt + 1])
                gates.append(gt)
            for chunk in range(NCHUNK):
                y0 = chunk * CHUNK_ROWS
                for i in range(2):
                    for r in range(2):
                        dst = xca[i][:].rearrange("p (y x) -> p y x", x=WP)
                        dst = dst[:, 1 + y0 + r:1 + y0 + 10:2, 1:41]
                        g = gates[chunk][:, i * CHUNK_N:(i + 1) * CHUNK_N]
                        g = g.rearrange("p (y x) -> p y x", x=40)[:, r::2]
                        nc.vector.tensor_tensor(dst, up_ap(2 + i, s, chunk), g, AluOpType.mult)
            return recs

        def stream_sa(wt):
            slabs = {}
            for mt in range(2):
                for kt in range(2):
                    tl = mp.tile([128, 1152], BF16, tag="wsbf", bufs=5)
                    nc.sync.dma_start(out=tl[:], in_=wt[kt, mt])
                    slabs[(kt, mt)] = tl
            return slabs

        def conv3x3(src_tiles, slab_fn, mt, kt_n, chunk):
            ps0 = psC.tile([128, 512], F32, tag="cps", name="cps")
            ps = ps0[:, 0:CHUNK_N]
            y0 = chunk * CHUNK_ROWS
            first = True
            for kt in range(kt_n):
                base = src_tiles[kt][:].rearrange("p (y x) -> p y x", x=WP)
                for dy in range(3):
                    for dx in range(3):
                        rhs = base[:, y0 + dy:y0 + dy + CHUNK_ROWS, dx:dx + 40]
                        lhsT = slab_fn(kt, mt, dy * 3 + dx)
                        nc.tensor.matmul(ps, lhsT, rhs, start=first,
                                         stop=(kt == kt_n - 1 and dy == 2 and dx == 2))
                        first = False
            return ps

        # ---------- main loop ----------
        vts, kk0 = stageA1(0)
        for t in attn + xca + xsa1:
            _apron_memset(nc, t)
        recs = stageA2(0, kk0)
        for s in range(B):
            # ---- SA conv1 ----
            sa1slabs = stream_sa(w['sa1wT'])
            for mt in range(2):
                for chunk in range(NCHUNK):
                    ps = conv3x3(xca, lambda kt, m, t: sa1slabs[(kt, m)][:, t * 128:(t + 1) * 128],
                                 mt, 2, chunk)[:, 0:CHUNK_N]
                    y0 = chunk * CHUNK_ROWS
                    dst = xsa1[mt][:].rearrange("p (y x) -> p y x", x=WP)[:, 1 + y0:11 + y0, 1:41]
                    nc.scalar.activation(dst, ps.rearrange("p (a b) -> p a b", b=40), AF.Silu,
                                         bias=vs['sa1b'][:, mt:mt + 1], scale=vs['sa1s'][:, mt:mt + 1])

            # ---- attention apply: 4 patch-positions per psum bank ----
            for h in range(8):
                expT, rbc = recs[h]
                o = (h % 2) * 64
                for py in range(4):
                    psy = psY.tile([128, 512], F32, tag="aps", name="aps")
                    for px in range(4):
                        pp = py * 4 + px
                        lhsT = vts[h // 2][0:100, pp * 128 + o: pp * 128 + o + 64]
                        nc.tensor.matmul(psy[o:o + 64, px * 100:px * 100 + 100], lhsT, expT[:],
                                         start=(px == 0), stop=(px == 3))
                    dstg = attn[h // 2][o:o + 64, :].rearrange("p (y x) -> p y x", x=WP)
                    dstg = dstg[:, 1 + py:38 + py:4, 1:41]
                    dstg = dstg.rearrange("p a (pwx px) -> p a pwx px", px=4)
                    in0 = psy[o:o + 64, 0:400].rearrange("p (px phy pwx) -> p phy pwx px",
                                                         px=4, phy=10)
                    in1 = rbc[o:o + 64, :].rearrange("p (a b) -> p a b", b=10)
                    in1 = in1.unsqueeze(3).broadcast_to([64, 10, 10, 4])
                    nc.vector.scalar_tensor_tensor(dstg, in0, 0.0, in1,
                                                   AluOpType.bypass, AluOpType.mult)

            # ---- prefetch next sample's A1 stage ----
            if s + 1 < B:
                next_vts, next_kk = stageA1(s + 1)

            # ---- SA conv2 + residual ----
            sa2slabs = stream_sa(w['sa2wT'])
            for mt in range(2):
                for chunk in range(NCHUNK):
                    ps = conv3x3(xsa1, lambda kt, m, t: sa2slabs[(kt, m)][:, t * 128:(t + 1) * 128],
                                 mt, 2, chunk)[:, 0:CHUNK_N]
                    y0 = chunk * CHUNK_ROWS
                    tsilu = mp.tile([128, CHUNK_N], F32, tag="silu", bufs=3)
                    nc.scalar.activation(tsilu[:], ps, AF.Silu,
                                         bias=vs['sa2b'][:, mt:mt + 1], scale=vs['sa2s'][:, mt:mt + 1])
                    xc = xca[mt][:].rearrange("p (y x) -> p y x", x=WP)[:, 1 + y0:11 + y0, 1:41]
                    nc.vector.tensor_tensor(a2[mt][:, y0 * 40:(y0 + 10) * 40],
                                            tsilu[:].rearrange("p (a b) -> p a b", b=40),
                                            xc, AluOpType.add)

            # ---- conv3 (resident bf16 slabs) + residual -> x2_out ----
            if s == 0:
                for mt in range(4):
                    for kt in range(4):
                        off = (mt * 4 + kt) * 1152
                        nc.sync.dma_start(out=c3_sb[:, off:off + 1152], in_=w['c3wT'][kt, mt])
            for mt in range(4):
                for chunk in range(NCHUNK):
                    ps = conv3x3(
                        attn,
                        lambda kt, m, t: c3_sb[:, (m * 4 + kt) * 1152 + t * 128:
                                               (m * 4 + kt) * 1152 + (t + 1) * 128],
                        mt, 4, chunk)[:, 0:CHUNK_N]
                    y0 = chunk * CHUNK_ROWS
                    tsilu = mp.tile([128, CHUNK_N], F32, tag="silu", bufs=3)
                    nc.scalar.activation(tsilu[:], ps, AF.Silu,
                                         bias=vs['c3b'][:, mt:mt + 1], scale=vs['c3s'][:, mt:mt + 1])
                    osb = mp.tile([128, CHUNK_N], F32, tag="osb", bufs=3)
                    at2 = attn[mt][:].rearrange("p (y x) -> p y x", x=WP)[:, 1 + y0:11 + y0, 1:41]
                    nc.vector.tensor_tensor(osb[:].rearrange("p (a b) -> p a b", b=40),
                                            tsilu[:].rearrange("p (a b) -> p a b", b=40),
                                            at2, AluOpType.add)
                    nc.sync.dma_start(
                        out=out.ap()[s, 512 + mt * 128:512 + (mt + 1) * 128]
                            .rearrange("p y x -> p (y x)")[:, y0 * 40:(y0 + 10) * 40],
                        in_=osb[:])

            # ---- prefetch next sample's A2 stage (scores/softmax + CA gate) ----
            if s + 1 < B:
                next_recs = stageA2(s + 1, next_kk)

            # ---- conv2 (1x1) -> x1_out ----
            for mt in range(4):
                for chunk in range(NCHUNK):
                    # ps layout: col = r*200 + y*40 + x  (pixel (2y+r)*40+x)
                    ps0 = psC.tile([128, 512], F32, tag="cps", name="cps")
                    ps = ps0[:, 0:CHUNK_N]
                    y0 = chunk * CHUNK_ROWS
                    first = True
                    for r in range(2):
                        for kt in range(4):
                            nc.tensor.matmul(
                                ps[:, r * 200:(r + 1) * 200],
                                c2_sb[:, (kt * 4 + mt) * 128:(kt * 4 + mt + 1) * 128],
                                up_ap(kt, s, chunk), start=first, stop=False)
                            first = False
                    for i in range(2):
                        kt = 4 + i
                        rhs = a2[i][:, y0 * 40:(y0 + 10) * 40]
                        rhs = rhs.rearrange("p (y r x) -> p r y x", y=5, r=2)
                        nc.tensor.matmul(ps, c2_sb[:, (kt * 4 + mt) * 128:(kt * 4 + mt + 1) * 128],
                                         rhs, start=False, stop=(i == 1))
                    osb = mp.tile([128, CHUNK_N], F32, tag="osb", bufs=3)
                    oview = osb[:].rearrange("p (y r x) -> p y r x", y=5, r=2)
                    pv = ps.rearrange("p (r y x) -> p y r x", r=2, y=5)
                    nc.scalar.activation(oview, pv, AF.Silu,
                                         bias=vs['c2b'][:, mt:mt + 1], scale=vs['c2s'][:, mt:mt + 1])
                    nc.sync.dma_start(
                        out=out.ap()[s, mt * 128:(mt + 1) * 128]
                            .rearrange("p y x -> p (y x)")[:, y0 * 40:(y0 + 10) * 40],
                        in_=osb[:])

            if s + 1 < B:
                vts, recs = next_vts, next_recs


# revision 60
# speedup vs baseline: 1.5650x; 1.1618x over previous
"""Trainium2 Bass kernel for nn_CSFAProV2 — full-input contract.

kernel(**inputs) takes the FULL unsharded inputs (B=32), shards the batch
across 8 NeuronCores (4 samples each, pure data parallel over axis 0 of
x1/x2, weights replicated), compiles+runs the Bass/Tile kernel via
run_bass_kernel_spmd, and concatenates the per-core outputs into the full
[32, 1024, 40, 40] result. Self-contained: the Bass kernel builder is
inlined below; only needs /opt/trn_rl_repo (concourse) + numpy/ml_dtypes.
"""

import sys

if '/opt/trn_rl_repo' not in sys.path:
    sys.path.insert(0, '/opt/trn_rl_repo')

import numpy as np

N_CORES = 8
B_FULL = 32
B_CORE = B_FULL // N_CORES

_cache = {}


def make_in_maps(inputs):
    import ml_dtypes
    bf16 = ml_dtypes.bfloat16
    wd = prep_weights(inputs)
    x1 = np.asarray(inputs['x1'], np.float32)
    x2 = np.asarray(inputs['x2'], np.float32)
    B = x2.shape[0]
    x1p = np.zeros((4, 128, B, 22, 22), np.float32)
    x1p[:, :, :, 1:21, 1:21] = x1.reshape(B, 4, 128, 20, 20).transpose(1, 2, 0, 3, 4)
    x1bf = x1p.reshape(4, 128, B, 484).astype(bf16)
    # host-side x2 staging: value slabs pre-transposed to [k, (pp, ch)] per
    # 128-channel group, and 4x4 patch sums for the key projection
    x = x2.reshape(B, 4, 128, 10, 4, 10, 4)           # s ct f ky py kx px
    vall = x.transpose(0, 1, 3, 5, 4, 6, 2)           # s ct ky kx py px f
    vall = np.ascontiguousarray(vall.reshape(B, 4, 100, 2048)).astype(bf16)
    kpd = np.ascontiguousarray(
        x.sum(axis=(4, 6)).reshape(B, 4, 128, 100)).astype(bf16)

    in_maps = []
    for c in range(N_CORES):
        m = dict(wd)
        m['x1bf'] = np.ascontiguousarray(
            x1bf[:, :, c * B_CORE:(c + 1) * B_CORE].reshape(4, 128, B_CORE * 484))
        m['vall'] = np.ascontiguousarray(vall[c * B_CORE:(c + 1) * B_CORE])
        m['kpd'] = np.ascontiguousarray(kpd[c * B_CORE:(c + 1) * B_CORE])
        in_maps.append(m)
    return in_maps


def kernel(**inputs):
    from concourse.bass_utils import run_bass_kernel_spmd

    if 'nc' not in _cache:
        _cache['nc'] = build_nc(B=B_CORE)
    nc = _cache['nc']

    in_maps = make_in_maps(inputs)
    res = run_bass_kernel_spmd(nc, in_maps, core_ids=list(range(N_CORES)))
    return np.concatenate([res.results[c]['out'] for c in range(N_CORES)], axis=0)


# ======================================================================
# Inlined Bass/Tile kernel builder
# ======================================================================

"""Per-core kernel, B batch samples.

Host pre-stages layouts (numpy): x1 zero-padded bf16, x2 value slabs
pre-transposed to [patch, (pixpos, ch)], 4x4 patch sums for the key proj.
Software-pipelined per sample: A1(s+1) = slab loads + key proj (issued
after SA2(s)); A2a(s+1) = scores+exp (after A1); A2b(s+1) = softmax sums +
CA gate + gate apply (issued mid-conv3(s)). The CA gate and conv2's x1
contraction run on the 20x20 grid (2x2-block constant under the nearest
upsample). All matmuls bf16; channel-on-partition; 3x3 convs = PE matmuls
accumulating over (ktile, tap) on padded-spatial SBUF tiles; conv3 weights
resident in SBUF.
"""

import math
import numpy as np
import concourse.bass as bass
import concourse.mybir as mybir
from concourse import bacc
from concourse import masks
from concourse.tile import TileContext
from concourse.alu_op_type import AluOpType

F32 = mybir.dt.float32
F32R = mybir.dt.float32r
BF16 = mybir.dt.bfloat16
FP8 = mybir.dt.float8e4
DR = mybir.MatmulPerfMode.DoubleRow
AF = mybir.ActivationFunctionType

H = W = 40
WP = 42
PADPIX = WP * WP
CHUNK_ROWS = 10
NCHUNK = H // CHUNK_ROWS
CHUNK_N = CHUNK_ROWS * W


def prep_weights(inp):
    import ml_dtypes
    bf16 = ml_dtypes.bfloat16
    d = {}

    def convT(w, kt_n, mt_n):  # [Cout, Cin, 3, 3] -> [kt, mt, 128, 9*128]
        x = np.asarray(w, np.float32).reshape(mt_n, 128, kt_n, 128, 9)
        x = x.transpose(2, 0, 3, 4, 1)
        return np.ascontiguousarray(x.reshape(kt_n, mt_n, 128, 9 * 128))

    d['qwT'] = convT(inp['q_w'], 4, 2).astype(bf16)
    d['c3wT'] = convT(inp['c3_w'], 4, 4).astype(bf16)
    d['sa1wT'] = convT(inp['sa1_w'], 2, 2).astype(bf16)
    d['sa2wT'] = convT(inp['sa2_w'], 2, 2).astype(bf16)
    w2 = np.asarray(inp['conv2_w'], np.float32)[:, :, 0, 0]
    d['c2wT'] = np.ascontiguousarray(
        w2.reshape(4, 128, 6, 128).transpose(2, 0, 3, 1)).astype(bf16)
    # the patch mean's /16 is folded into the key projection
    kw = np.asarray(inp['key_w'], np.float32) * 0.0625
    d['keywT'] = np.ascontiguousarray(
        kw.reshape(2, 128, 4, 128).transpose(2, 0, 3, 1)).astype(bf16)
    w1 = np.asarray(inp['ca_w1'], np.float32)
    d['caw1T'] = np.ascontiguousarray(
        w1.reshape(64, 2, 128).transpose(1, 2, 0)).astype(bf16)
    w2c = np.asarray(inp['ca_w2'], np.float32)
    d['caw2T'] = np.ascontiguousarray(
        w2c.reshape(2, 128, 64).transpose(2, 0, 1)).astype(bf16)
    for nm, key in [('qs', 'q_s'), ('qb', 'q_b'), ('c3s', 'c3_s'), ('c3b', 'c3_b'),
                    ('sa1s', 'sa1_s'), ('sa1b', 'sa1_b'), ('sa2s', 'sa2_s'),
                    ('sa2b', 'sa2_b'), ('c2s', 'conv2_s'), ('c2b', 'conv2_b'),
                    ('cab1', 'ca_b1'), ('cab2', 'ca_b2')]:
        d[nm] = np.ascontiguousarray(np.asarray(inp[key], np.float32))
    return d


def build_nc(B=4):
    nc = bacc.Bacc(None)
    w = {}
    w['x1bf'] = nc.dram_tensor("x1bf", [4, 128, B * 484], BF16, kind="ExternalInput")
    w['vall'] = nc.dram_tensor("vall", [B, 4, 100, 2048], BF16, kind="ExternalInput")
    w['kpd'] = nc.dram_tensor("kpd", [B, 4, 128, 100], BF16, kind="ExternalInput")
    w['qwT'] = nc.dram_tensor("qwT", [4, 2, 128, 9 * 128], BF16, kind="ExternalInput")
    w['c3wT'] = nc.dram_tensor("c3wT", [4, 4, 128, 9 * 128], BF16, kind="ExternalInput")
    w['sa1wT'] = nc.dram_tensor("sa1wT", [2, 2, 128, 9 * 128], BF16, kind="ExternalInput")
    w['sa2wT'] = nc.dram_tensor("sa2wT", [2, 2, 128, 9 * 128], BF16, kind="ExternalInput")
    w['c2wT'] = nc.dram_tensor("c2wT", [6, 4, 128, 128], BF16, kind="ExternalInput")
    w['keywT'] = nc.dram_tensor("keywT", [4, 2, 128, 128], BF16, kind="ExternalInput")
    w['caw1T'] = nc.dram_tensor("caw1T", [2, 128, 64], BF16, kind="ExternalInput")
    w['caw2T'] = nc.dram_tensor("caw2T", [64, 2, 128], BF16, kind="ExternalInput")
    for nm, n in [('qs', 256), ('qb', 256), ('c3s', 512), ('c3b', 512),
                  ('sa1s', 256), ('sa1b', 256), ('sa2s', 256), ('sa2b', 256),
                  ('c2s', 512), ('c2b', 512), ('cab1', 64), ('cab2', 256)]:
        w[nm] = nc.dram_tensor(nm, [n], F32, kind="ExternalInput")
    out = nc.dram_tensor("out", [B, 1024, 40, 40], F32, kind="ExternalOutput")

    with TileContext(nc) as tc:
        _emit(nc, tc, B, w, out)
    nc.finalize()
    return nc


def _apron_memset(nc, t):
    nc.gpsimd.memset(t[:, 0:WP], 0.0)
    nc.gpsimd.memset(t[:, 41 * WP:42 * WP], 0.0)
    g = t[:].rearrange("p (y x) -> p y x", x=WP)
    nc.gpsimd.memset(g[:, 1:41, 0:1], 0.0)
    nc.gpsimd.memset(g[:, 1:41, 41:42], 0.0)


def _emit(nc, tc, B, w, out):
    import contextlib
    ctx = contextlib.ExitStack()
    with ctx:
        mp = ctx.enter_context(tc.tile_pool(name="main", bufs=1))
        psC = ctx.enter_context(tc.tile_pool(name="psC", bufs=3, space="PSUM"))
        psY = ctx.enter_context(tc.tile_pool(name="psY", bufs=5, space="PSUM"))

        # ---------- startup: x1 + q-conv weights, interleaved so the first
        # matmul's deps (x1pad ct0 + qslab(0,0)) land first ----------
        x1pad = []
        qslabs = {}
        for ct in range(4):
            t = mp.tile([128, B * 484], BF16, tag=f"x1pad{ct}")
            nc.sync.dma_start(out=t[:], in_=w['x1bf'].ap()[ct])
            x1pad.append(t)
            qbf = mp.tile([128, 1152], BF16, tag="wsbf", bufs=8)
            nc.sync.dma_start(out=qbf[:], in_=w['qwT'][ct, 0])
            qslabs[(ct, 0)] = qbf
        # key-proj weights next: stageA1(0)'s kproj deps must not queue
        # behind the second qconv slab batch
        keyw_sb = mp.tile([128, 8 * 128], BF16, tag="keyw")
        for kt in range(4):
            nc.sync.dma_start(
                out=keyw_sb[:, kt * 256:(kt + 1) * 256].rearrange("p (m c) -> p m c", m=2),
                in_=w['keywT'][kt].rearrange("m p c -> p m c"))
        # sample-0 A1 loads issued here so kproj(0)/attn(0) deps land early
        kps0 = []
        for ct in range(4):
            kpt = mp.tile([128, 100], BF16, tag="kp", bufs=8)
            nc.sync.dma_start(out=kpt[:], in_=w['kpd'].ap()[0, ct])
            kps0.append(kpt)
        for kt in range(4):
            qbf = mp.tile([128, 1152], BF16, tag="wsbf", bufs=8)
            nc.sync.dma_start(out=qbf[:], in_=w['qwT'][kt, 1])
            qslabs[(kt, 1)] = qbf
        vts0 = []
        for ct in range(4):
            vt = mp.tile([128, 2048], BF16, tag="v", bufs=6)
            nc.sync.dma_start(out=vt[0:100, :], in_=w['vall'].ap()[0, ct])
            vts0.append(vt)

        def load_vec(name, n):
            p = min(n, 128)
            t = mp.tile([128, max(n // 128, 1)], F32, tag=f"vec_{name}")
            nc.sync.dma_start(out=t[0:p, 0:max(n // 128, 1)],
                              in_=w[name].ap().rearrange("(a p) -> p a", p=p))
            return t
        vs = {}
        for nm in ('qs', 'qb'):
            vs[nm] = load_vec(nm, 256)



        ones_bf = mp.tile([128, 1], BF16, tag="ones")
        nc.gpsimd.memset(ones_bf[:], 1.0)

        def x1_s(ct, s):
            base = x1pad[ct][:].rearrange("p (ss a) -> p ss a", ss=B)[:, s]
            return base.rearrange("p (y x) -> p y x", x=22)[:, 1:21, 1:21]

        # ---------- q conv ----------
        q_sb = mp.tile([128, 2 * B * 100], BF16, tag="qsb")
        for mt in range(2):
            ps0 = psC.tile([128, 512], F32, tag="cps", name="cps")
            ps = ps0[:, 0:B * 100]
            first = True
            for kt in range(4):
                base = x1pad[kt][:].rearrange("p (s y x) -> p s y x", s=B, x=22)
                for dy in range(3):
                    for dx in range(3):
                        rhs = base[:, :, dy:dy + 20:2, dx:dx + 20:2]
                        nc.tensor.matmul(
                            ps, qslabs[(kt, mt)][:, (dy * 3 + dx) * 128:(dy * 3 + dx + 1) * 128],
                            rhs, start=first, stop=(kt == 3 and dy == 2 and dx == 2))
                        first = False
            nc.scalar.activation(q_sb[:, mt * B * 100:(mt + 1) * B * 100], ps, AF.Silu,
                                 bias=vs['qb'][:, mt:mt + 1], scale=vs['qs'][:, mt:mt + 1])

        caw1_sb = caw2_sb = c2_sb = None
        sa_sb = {}
        def load_late_weights():
            nonlocal caw1_sb, caw2_sb, c2_sb
            caw1_sb = mp.tile([128, 128], BF16, tag="caw1")
            for kt in range(2):
                nc.sync.dma_start(out=caw1_sb[:, kt * 64:(kt + 1) * 64], in_=w['caw1T'][kt])
            caw2_sb = mp.tile([64, 256], BF16, tag="caw2")
            nc.sync.dma_start(out=caw2_sb[:], in_=w['caw2T'].rearrange("p m c -> p (m c)"))
            for nm, n in [('cab1', 64), ('cab2', 256), ('c3s', 512), ('c3b', 512),
                          ('sa1s', 256), ('sa1b', 256), ('sa2s', 256), ('sa2b', 256),
                          ('c2s', 512), ('c2b', 512)]:
                vs[nm] = load_vec(nm, n)
            c2_sb = mp.tile([128, 6 * 512], BF16, tag="c2w")
            for kt in range(6):
                nc.sync.dma_start(
                    out=c2_sb[:, kt * 512:(kt + 1) * 512].rearrange("p (m c) -> p m c", m=4),
                    in_=w['c2wT'][kt].rearrange("m p c -> p m c"))
            for nm in ('sa1wT', 'sa2wT'):
                for mt in range(2):
                    for kt in range(2):
                        tl = mp.tile([128, 1152], BF16, tag=f"sa_{nm}_{kt}_{mt}")
                        nc.sync.dma_start(out=tl[:], in_=w[nm][kt, mt])
                        sa_sb[(nm, kt, mt)] = tl

        # conv3 weights resident (16 slabs x [128, 1152] bf16); DMAs issued
        # inside the loop at s==0 to keep them off the warmup DMA burst
        c3_sb = mp.tile([128, 16 * 1152], BF16, tag="c3w")

        SCALE = 1.0 / math.sqrt(32)

        # persistent big tensors: aprons zeroed once, interiors rewritten
        attn = []
        for ct in range(4):
            at = mp.tile([128, PADPIX], BF16, tag=f"attn{ct}", name="at", bufs=1)
            attn.append(at)
        xca = []
        xsa1 = []
        a2 = []
        for i in range(2):
            t = mp.tile([128, PADPIX], BF16, tag=f"xca{i}", name="t", bufs=1)
            xca.append(t)
            t2 = mp.tile([128, PADPIX], BF16, tag=f"xsa{i}", name="t2", bufs=1)
            xsa1.append(t2)
            a2t = mp.tile([128, 1600], BF16, tag=f"a2_{i}", name="a2t", bufs=1)
            a2.append(a2t)


        # ---------- stage A1: load host-staged value slabs + key-proj ----------
        def loadA1_kp(s):
            kps = []
            for ct in range(4):
                kpt = mp.tile([128, 100], BF16, tag="kp", bufs=8)
                nc.sync.dma_start(out=kpt[:], in_=w['kpd'].ap()[s, ct])
                kps.append(kpt)
            return kps

        def loadA1_v(s):
            vts = []
            for ct in range(4):
                vt = mp.tile([128, 2048], BF16, tag="v", bufs=6)
                nc.sync.dma_start(out=vt[0:100, :], in_=w['vall'].ap()[s, ct])
                vts.append(vt)
            return vts

        def stageA1(s, pre=None):
            kps, vts = pre if pre is not None else (loadA1_kp(s), loadA1_v(s))
            # key projection
            kk = mp.tile([128, 200], BF16, tag="ksb", bufs=2)
            for mt in range(2):
                psk = psY.tile([128, 512], F32, tag="aps", name="aps")
                for kt in range(4):
                    nc.tensor.matmul(
                        psk[:, 0:100], keyw_sb[:, (kt * 2 + mt) * 128:(kt * 2 + mt + 1) * 128],
                        kps[kt][:], start=(kt == 0), stop=(kt == 3))
                nc.vector.tensor_copy(kk[:, mt * 100:(mt + 1) * 100], psk[:, 0:100])
            return vts, kk

        # ---------- stage A2a: attention scores + exp ----------
        def stageA2a(s, kk):
            exps = []
            for h in range(8):
                emb_ct, emb_off = h // 4, (h % 4) * 32
                pssc = psY.tile([128, 512], F32, tag="aps", name="aps")
                lhs = kk[emb_off:emb_off + 32, emb_ct * 100:(emb_ct + 1) * 100]
                rhs = q_sb[emb_off:emb_off + 32, (emb_ct * B + s) * 100:(emb_ct * B + s + 1) * 100]
                nc.tensor.matmul(pssc[0:100, 0:100], lhs, rhs, tile_position=(emb_off, 0))
                expT = mp.tile([100, 100], BF16, tag="expT", bufs=10)
                nc.scalar.activation(expT[:], pssc[0:100, 0:100], AF.Exp, scale=SCALE)
                exps.append(expT)
            return exps

        # ---------- stage A2b: softmax sums + CA gate + gate apply ----------
        def stageA2b(s, exps):
            recs = []
            for quad in range(2):
                pssum = psY.tile([128, 512], F32, tag="aps", name="aps")
                for hh in range(4):
                    nc.tensor.matmul(pssum[0:1, hh * 100:(hh + 1) * 100],
                                     ones_bf[0:100, 0:1], exps[quad * 4 + hh][:])
                recip4 = mp.tile([1, 400], F32, tag="recip", bufs=2)
                nc.vector.reciprocal(recip4[:], pssum[0:1, 0:400])
                for hh in range(4):
                    rbc = mp.tile([128, 100], F32, tag="rbc", bufs=10)
                    nc.gpsimd.partition_broadcast(rbc[:], recip4[0:1, hh * 100:(hh + 1) * 100])
                    recs.append((exps[quad * 4 + hh], rbc))
            # CA gate at 20x20: the gate logits are 2x2-block constant under
            # the nearest-neighbor upsample, so compute the MLP on the 20x20
            # grid (4x fewer MACs) and broadcast in the apply multiply.
            hps = psC.tile([64, 512], F32, tag="cps", name="cps")
            for i in range(2):
                nc.tensor.matmul(hps[:, 0:400], caw1_sb[:, i * 64:(i + 1) * 64],
                                 x1_s(2 + i, s), start=(i == 0), stop=(i == 1))
            hsb = mp.tile([64, 400], BF16, tag="hsb", bufs=2)
            nc.scalar.activation(hsb[:], hps[:, 0:400], AF.Relu, bias=vs['cab1'][0:64, 0:1])
            gts = []
            for mt in range(2):
                gps = psC.tile([128, 512], F32, tag="cps", name="cps")
                nc.tensor.matmul(gps[:, 0:400], caw2_sb[0:64, mt * 128:(mt + 1) * 128], hsb[:])
                gt = mp.tile([128, 400], BF16, tag="gate", bufs=4)
                nc.scalar.activation(gt[:], gps[:, 0:400], AF.Sigmoid,
                                     bias=vs['cab2'][:, mt:mt + 1])
                gts.append(gt)
            for i in range(2):
                dst = xca[i][:].rearrange("p (y x) -> p y x", x=WP)[:, 1:41, 1:41]
                dst5 = dst.rearrange("p (yy r) (xx px) -> p yy r xx px", r=2, px=2)
                a14 = x1_s(2 + i, s).unsqueeze(3).broadcast_to([128, 20, 20, 2])
                g4 = gts[i][:].rearrange("p (y x) -> p y x", x=20)
                g4 = g4.unsqueeze(3).broadcast_to([128, 20, 20, 2])
                for r in range(2):
                    nc.vector.tensor_tensor(dst5[:, :, r], a14, g4, AluOpType.mult)
            return recs

        def conv3x3_mt(src_tiles, slab_fn, mt, kt_n):
            pss = []
            for chunk in range(NCHUNK):
                ps0 = psC.tile([128, 512], F32, tag="cps", name="cps")
                y0 = chunk * CHUNK_ROWS
                first = True
                for kt in range(kt_n):
                    base = src_tiles[kt][:].rearrange("p (y x) -> p y x", x=WP)
                    for dy in range(3):
                        for dx in range(3):
                            rhs = base[:, y0 + dy:y0 + dy + CHUNK_ROWS, dx:dx + 40]
                            lhsT = slab_fn(kt, mt, dy * 3 + dx)
                            nc.tensor.matmul(ps0[:, 0:CHUNK_N], lhsT, rhs, start=first,
                                             stop=(kt == kt_n - 1 and dy == 2 and dx == 2))
                            first = False
                pss.append(ps0)
            return pss

        # ---------- main loop ----------
        vts, kk0 = stageA1(0, pre=(kps0, vts0))
        exps0 = stageA2a(0, kk0)
        load_late_weights()
        for t in attn + xca + xsa1:
            _apron_memset(nc, t)
        recs = stageA2b(0, exps0)
        for s in range(B):
            # ---- SA conv1 ----
            for mt in range(2):
                pss = conv3x3_mt(
                    xca, lambda kt, m, t: sa_sb[('sa1wT', kt, m)][:, t * 128:(t + 1) * 128],
                    mt, 2)
                for chunk in range(NCHUNK):
                    ps = pss[chunk][:, 0:CHUNK_N]
                    y0 = chunk * CHUNK_ROWS
                    dst = xsa1[mt][:].rearrange("p (y x) -> p y x", x=WP)[:, 1 + y0:11 + y0, 1:41]
                    nc.scalar.activation(dst, ps.rearrange("p (a b) -> p a b", b=40), AF.Silu,
                                         bias=vs['sa1b'][:, mt:mt + 1], scale=vs['sa1s'][:, mt:mt + 1])

            # ---- attention apply: 4 patch-positions per psum bank ----
            for h in range(8):
                expT, rbc = recs[h]
                o = (h % 2) * 64
                for py in range(4):
                    psy = psY.tile([128, 512], F32, tag="aps", name="aps")
                    for px in range(4):
                        pp = py * 4 + px
                        lhsT = vts[h // 2][0:100, pp * 128 + o: pp * 128 + o + 64]
                        nc.tensor.matmul(psy[o:o + 64, px * 100:px * 100 + 100], lhsT, expT[:],
                                         start=(px == 0), stop=(px == 3))
                    dstg = attn[h // 2][o:o + 64, :].rearrange("p (y x) -> p y x", x=WP)
                    dstg = dstg[:, 1 + py:38 + py:4, 1:41]
                    dstg = dstg.rearrange("p a (pwx px) -> p a pwx px", px=4)
                    in0 = psy[o:o + 64, 0:400].rearrange("p (px phy pwx) -> p phy pwx px",
                                                         px=4, phy=10)
                    in1 = rbc[o:o + 64, :].rearrange("p (a b) -> p a b", b=10)
                    in1 = in1.unsqueeze(3).broadcast_to([64, 10, 10, 4])
                    nc.vector.scalar_tensor_tensor(dstg, in0, 0.0, in1,
                                                   AluOpType.bypass, AluOpType.mult)

            # ---- prefetch next sample's A1 stage + scores/exp ----
            if s + 1 < B:
                next_vts, next_kk = stageA1(s + 1)
                next_exps = stageA2a(s + 1, next_kk)

            # ---- SA conv2 + residual ----
            for mt in range(2):
                pss = conv3x3_mt(
                    xsa1, lambda kt, m, t: sa_sb[('sa2wT', kt, m)][:, t * 128:(t + 1) * 128],
                    mt, 2)
                for chunk in range(NCHUNK):
                    ps = pss[chunk][:, 0:CHUNK_N]
                    y0 = chunk * CHUNK_ROWS
                    tsilu = mp.tile([128, CHUNK_N], F32, tag="silu", bufs=3)
                    nc.scalar.activation(tsilu[:], ps, AF.Silu,
                                         bias=vs['sa2b'][:, mt:mt + 1], scale=vs['sa2s'][:, mt:mt + 1])
                    xc = xca[mt][:].rearrange("p (y x) -> p y x", x=WP)[:, 1 + y0:11 + y0, 1:41]
                    nc.vector.tensor_tensor(a2[mt][:, y0 * 40:(y0 + 10) * 40],
                                            tsilu[:].rearrange("p (a b) -> p a b", b=40),
                                            xc, AluOpType.add)

            # ---- conv3 (resident bf16 slabs) + residual -> x2_out ----
            if s == 0:
                for mt in range(4):
                    for kt in range(4):
                        off = (mt * 4 + kt) * 1152
                        nc.sync.dma_start(out=c3_sb[:, off:off + 1152], in_=w['c3wT'][kt, mt])
            for mt in range(2):
                pss = conv3x3_mt(
                    attn,
                    lambda kt, m, t: c3_sb[:, (m * 4 + kt) * 1152 + t * 128:
                                           (m * 4 + kt) * 1152 + (t + 1) * 128],
                    mt, 4)
                for chunk in range(NCHUNK):
                    ps = pss[chunk][:, 0:CHUNK_N]
                    y0 = chunk * CHUNK_ROWS
                    tsilu = mp.tile([128, CHUNK_N], F32, tag="silu", bufs=3)
                    nc.scalar.activation(tsilu[:], ps, AF.Silu,
                                         bias=vs['c3b'][:, mt:mt + 1], scale=vs['c3s'][:, mt:mt + 1])
                    osb = mp.tile([128, CHUNK_N], F32, tag="osb", bufs=3)
                    at2 = attn[mt][:].rearrange("p (y x) -> p y x", x=WP)[:, 1 + y0:11 + y0, 1:41]
                    nc.vector.tensor_tensor(osb[:].rearrange("p (a b) -> p a b", b=40),
                                            tsilu[:].rearrange("p (a b) -> p a b", b=40),
                                            at2, AluOpType.add)
                    nc.sync.dma_start(
                        out=out.ap()[s, 512 + mt * 128:512 + (mt + 1) * 128]
                            .rearrange("p y x -> p (y x)")[:, y0 * 40:(y0 + 10) * 40],
                        in_=osb[:])

            # ---- prefetch next sample's softmax sums + CA gate mid-conv3 ----
            if s + 1 < B:
                next_recs = stageA2b(s + 1, next_exps)

            for mt in range(2, 4):
                pss = conv3x3_mt(
                    attn,
                    lambda kt, m, t: c3_sb[:, (m * 4 + kt) * 1152 + t * 128:
                                           (m * 4 + kt) * 1152 + (t + 1) * 128],
                    mt, 4)
                for chunk in range(NCHUNK):
                    ps = pss[chunk][:, 0:CHUNK_N]
                    y0 = chunk * CHUNK_ROWS
                    tsilu = mp.tile([128, CHUNK_N], F32, tag="silu", bufs=3)
                    nc.scalar.activation(tsilu[:], ps, AF.Silu,
                                         bias=vs['c3b'][:, mt:mt + 1], scale=vs['c3s'][:, mt:mt + 1])
                    osb = mp.tile([128, CHUNK_N], F32, tag="osb", bufs=3)
                    at2 = attn[mt][:].rearrange("p (y x) -> p y x", x=WP)[:, 1 + y0:11 + y0, 1:41]
                    nc.vector.tensor_tensor(osb[:].rearrange("p (a b) -> p a b", b=40),
                                            tsilu[:].rearrange("p (a b) -> p a b", b=40),
                                            at2, AluOpType.add)
                    nc.sync.dma_start(
                        out=out.ap()[s, 512 + mt * 128:512 + (mt + 1) * 128]
                            .rearrange("p y x -> p (y x)")[:, y0 * 40:(y0 + 10) * 40],
                        in_=osb[:])

            # ---- conv2 (1x1) -> x1_out ----
            # x1pad channels are 2x2-block constant: contract them once per mt
            # on the 20x20 grid, then add (broadcast) to the 40x40 a2 part.
            for mt in range(4):
                psx = psY.tile([128, 512], F32, tag="aps", name="aps")
                for kt in range(4):
                    nc.tensor.matmul(psx[:, 0:400],
                                     c2_sb[:, (kt * 4 + mt) * 128:(kt * 4 + mt + 1) * 128],
                                     x1_s(kt, s), start=(kt == 0), stop=(kt == 3))
                x1c = mp.tile([128, 400], F32, tag="x1c", bufs=2)
                nc.vector.tensor_copy(x1c[:], psx[:, 0:400])
                for chunk in range(NCHUNK):
                    ps0 = psC.tile([128, 512], F32, tag="cps", name="cps")
                    ps = ps0[:, 0:CHUNK_N]
                    y0 = chunk * CHUNK_ROWS
                    for i in range(2):
                        rhs = a2[i][:, y0 * 40:(y0 + 10) * 40]
                        nc.tensor.matmul(ps, c2_sb[:, ((4 + i) * 4 + mt) * 128:((4 + i) * 4 + mt + 1) * 128],
                                         rhs, start=(i == 0), stop=(i == 1))
                    tsum = mp.tile([128, CHUNK_N], F32, tag="silu", bufs=3)
                    t5 = tsum[:].rearrange("p (yy r xx px) -> p yy r xx px", r=2, xx=20, px=2)
                    p5 = ps.rearrange("p (yy r xx px) -> p yy r xx px", r=2, xx=20, px=2)
                    xc4 = x1c[:, chunk * 100:(chunk + 1) * 100].rearrange("p (y x) -> p y x", x=20)
                    xc4 = xc4.unsqueeze(3).broadcast_to([128, 5, 20, 2])
                    for r in range(2):
                        nc.vector.tensor_tensor(t5[:, :, r], p5[:, :, r], xc4, AluOpType.add)
                    osb = mp.tile([128, CHUNK_N], F32, tag="osb", bufs=3)
                    nc.scalar.activation(osb[:], tsum[:], AF.Silu,
                                         bias=vs['c2b'][:, mt:mt + 1], scale=vs['c2s'][:, mt:mt + 1])
                    nc.sync.dma_start(
                        out=out.ap()[s, mt * 128:(mt + 1) * 128]
                            .rearrange("p y x -> p (y x)")[:, y0 * 40:(y0 + 10) * 40],
                        in_=osb[:])

            if s + 1 < B:
                vts, recs = next_vts, next_recs


# revision 61
# speedup vs baseline: 1.5912x; 1.0167x over previous
"""Trainium2 Bass kernel for nn_CSFAProV2 — full-input contract.

kernel(**inputs) takes the FULL unsharded inputs (B=32), shards the batch
across 8 NeuronCores (4 samples each, pure data parallel over axis 0 of
x1/x2, weights replicated), compiles+runs the Bass/Tile kernel via
run_bass_kernel_spmd, and concatenates the per-core outputs into the full
[32, 1024, 40, 40] result. Self-contained: the Bass kernel builder is
inlined below; only needs /opt/trn_rl_repo (concourse) + numpy/ml_dtypes.
"""

import sys

if '/opt/trn_rl_repo' not in sys.path:
    sys.path.insert(0, '/opt/trn_rl_repo')

import numpy as np

N_CORES = 8
B_FULL = 32
B_CORE = B_FULL // N_CORES

_cache = {}


def make_in_maps(inputs):
    import ml_dtypes
    bf16 = ml_dtypes.bfloat16
    wd = prep_weights(inputs)
    x1 = np.asarray(inputs['x1'], np.float32)
    x2 = np.asarray(inputs['x2'], np.float32)
    B = x2.shape[0]
    x1p = np.zeros((4, 128, B, 22, 22), np.float32)
    x1p[:, :, :, 1:21, 1:21] = x1.reshape(B, 4, 128, 20, 20).transpose(1, 2, 0, 3, 4)
    x1bf = x1p.reshape(4, 128, B, 484).astype(bf16)
    # host-side x2 staging: value slabs pre-transposed to [k, (pp, ch)] per
    # 128-channel group, and 4x4 patch sums for the key projection
    x = x2.reshape(B, 4, 128, 10, 4, 10, 4)           # s ct f ky py kx px
    vall = x.transpose(0, 1, 3, 5, 4, 6, 2)           # s ct ky kx py px f
    vall = np.ascontiguousarray(vall.reshape(B, 4, 100, 2048)).astype(bf16)
    kpd = np.ascontiguousarray(
        x.sum(axis=(4, 6)).reshape(B, 4, 128, 100)).astype(bf16)

    in_maps = []
    for c in range(N_CORES):
        m = dict(wd)
        m['x1bf'] = np.ascontiguousarray(
            x1bf[:, :, c * B_CORE:(c + 1) * B_CORE].reshape(4, 128, B_CORE * 484))
        m['vall'] = np.ascontiguousarray(vall[c * B_CORE:(c + 1) * B_CORE])
        m['kpd'] = np.ascontiguousarray(kpd[c * B_CORE:(c + 1) * B_CORE])
        in_maps.append(m)
    return in_maps


def kernel(**inputs):
    from concourse.bass_utils import run_bass_kernel_spmd

    if 'nc' not in _cache:
        _cache['nc'] = build_nc(B=B_CORE)
    nc = _cache['nc']

    in_maps = make_in_maps(inputs)
    res = run_bass_kernel_spmd(nc, in_maps, core_ids=list(range(N_CORES)))
    return np.concatenate([res.results[c]['out'] for c in range(N_CORES)], axis=0)


# ======================================================================
# Inlined Bass/Tile kernel builder
# ======================================================================

"""Per-core kernel, B batch samples.

Host pre-stages layouts (numpy): x1 zero-padded bf16, x2 value slabs
pre-transposed to [patch, (pixpos, ch)], 4x4 patch sums for the key proj.
Software-pipelined per sample: A1(s+1) = slab loads + key proj (issued
after SA2(s)); A2a(s+1) = scores+exp (after A1); A2b(s+1) = softmax sums +
CA gate + gate apply (issued mid-conv3(s)). The CA gate and conv2's x1
contraction run on the 20x20 grid (2x2-block constant under the nearest
upsample). All matmuls bf16; channel-on-partition; 3x3 convs = PE matmuls
accumulating over (ktile, tap) on padded-spatial SBUF tiles; conv3 weights
resident in SBUF.
"""

import math
import numpy as np
import concourse.bass as bass
import concourse.mybir as mybir
from concourse import bacc
from concourse import masks
from concourse.tile import TileContext
from concourse.alu_op_type import AluOpType

F32 = mybir.dt.float32
F32R = mybir.dt.float32r
BF16 = mybir.dt.bfloat16
FP8 = mybir.dt.float8e4
DR = mybir.MatmulPerfMode.DoubleRow
AF = mybir.ActivationFunctionType

H = W = 40
WP = 42
PADPIX = WP * WP
CHUNK_ROWS = 10
NCHUNK = H // CHUNK_ROWS
CHUNK_N = CHUNK_ROWS * W


def prep_weights(inp):
    import ml_dtypes
    bf16 = ml_dtypes.bfloat16
    d = {}

    def convT(w, kt_n, mt_n):  # [Cout, Cin, 3, 3] -> [kt, mt, 128, 9*128]
        x = np.asarray(w, np.float32).reshape(mt_n, 128, kt_n, 128, 9)
        x = x.transpose(2, 0, 3, 4, 1)
        return np.ascontiguousarray(x.reshape(kt_n, mt_n, 128, 9 * 128))

    d['qwT'] = convT(inp['q_w'], 4, 2).astype(bf16)
    d['c3wT'] = convT(inp['c3_w'], 4, 4).astype(bf16)
    d['sa1wT'] = convT(inp['sa1_w'], 2, 2).astype(bf16)
    d['sa2wT'] = convT(inp['sa2_w'], 2, 2).astype(bf16)
    w2 = np.asarray(inp['conv2_w'], np.float32)[:, :, 0, 0]
    d['c2wT'] = np.ascontiguousarray(
        w2.reshape(4, 128, 6, 128).transpose(2, 0, 3, 1)).astype(bf16)
    # the patch mean's /16 is folded into the key projection
    kw = np.asarray(inp['key_w'], np.float32) * 0.0625
    d['keywT'] = np.ascontiguousarray(
        kw.reshape(2, 128, 4, 128).transpose(2, 0, 3, 1)).astype(bf16)
    w1 = np.asarray(inp['ca_w1'], np.float32)
    d['caw1T'] = np.ascontiguousarray(
        w1.reshape(64, 2, 128).transpose(1, 2, 0)).astype(bf16)
    w2c = np.asarray(inp['ca_w2'], np.float32)
    d['caw2T'] = np.ascontiguousarray(
        w2c.reshape(2, 128, 64).transpose(2, 0, 1)).astype(bf16)
    for nm, key in [('qs', 'q_s'), ('qb', 'q_b'), ('c3s', 'c3_s'), ('c3b', 'c3_b'),
                    ('sa1s', 'sa1_s'), ('sa1b', 'sa1_b'), ('sa2s', 'sa2_s'),
                    ('sa2b', 'sa2_b'), ('c2s', 'conv2_s'), ('c2b', 'conv2_b'),
                    ('cab1', 'ca_b1'), ('cab2', 'ca_b2')]:
        d[nm] = np.ascontiguousarray(np.asarray(inp[key], np.float32))
    return d


def build_nc(B=4):
    nc = bacc.Bacc(None)
    w = {}
    w['x1bf'] = nc.dram_tensor("x1bf", [4, 128, B * 484], BF16, kind="ExternalInput")
    w['vall'] = nc.dram_tensor("vall", [B, 4, 100, 2048], BF16, kind="ExternalInput")
    w['kpd'] = nc.dram_tensor("kpd", [B, 4, 128, 100], BF16, kind="ExternalInput")
    w['qwT'] = nc.dram_tensor("qwT", [4, 2, 128, 9 * 128], BF16, kind="ExternalInput")
    w['c3wT'] = nc.dram_tensor("c3wT", [4, 4, 128, 9 * 128], BF16, kind="ExternalInput")
    w['sa1wT'] = nc.dram_tensor("sa1wT", [2, 2, 128, 9 * 128], BF16, kind="ExternalInput")
    w['sa2wT'] = nc.dram_tensor("sa2wT", [2, 2, 128, 9 * 128], BF16, kind="ExternalInput")
    w['c2wT'] = nc.dram_tensor("c2wT", [6, 4, 128, 128], BF16, kind="ExternalInput")
    w['keywT'] = nc.dram_tensor("keywT", [4, 2, 128, 128], BF16, kind="ExternalInput")
    w['caw1T'] = nc.dram_tensor("caw1T", [2, 128, 64], BF16, kind="ExternalInput")
    w['caw2T'] = nc.dram_tensor("caw2T", [64, 2, 128], BF16, kind="ExternalInput")
    for nm, n in [('qs', 256), ('qb', 256), ('c3s', 512), ('c3b', 512),
                  ('sa1s', 256), ('sa1b', 256), ('sa2s', 256), ('sa2b', 256),
                  ('c2s', 512), ('c2b', 512), ('cab1', 64), ('cab2', 256)]:
        w[nm] = nc.dram_tensor(nm, [n], F32, kind="ExternalInput")
    out = nc.dram_tensor("out", [B, 1024, 40, 40], F32, kind="ExternalOutput")

    with TileContext(nc) as tc:
        _emit(nc, tc, B, w, out)
    nc.finalize()
    return nc


def _apron_memset(nc, t):
    nc.gpsimd.memset(t[:, 0:WP], 0.0)
    nc.gpsimd.memset(t[:, 41 * WP:42 * WP], 0.0)
    g = t[:].rearrange("p (y x) -> p y x", x=WP)
    nc.gpsimd.memset(g[:, 1:41, 0:1], 0.0)
    nc.gpsimd.memset(g[:, 1:41, 41:42], 0.0)


def _emit(nc, tc, B, w, out):
    import contextlib
    ctx = contextlib.ExitStack()
    with ctx:
        mp = ctx.enter_context(tc.tile_pool(name="main", bufs=1))
        psC = ctx.enter_context(tc.tile_pool(name="psC", bufs=3, space="PSUM"))
        psY = ctx.enter_context(tc.tile_pool(name="psY", bufs=5, space="PSUM"))

        # ---------- startup: x1 + q-conv weights, interleaved so the first
        # matmul's deps (x1pad ct0 + qslab(0,0)) land first ----------
        x1pad = []
        qslabs = {}
        for ct in range(4):
            t = mp.tile([128, B * 484], BF16, tag=f"x1pad{ct}")
            nc.sync.dma_start(out=t[:], in_=w['x1bf'].ap()[ct])
            x1pad.append(t)
            qbf = mp.tile([128, 1152], BF16, tag="wsbf", bufs=8)
            nc.sync.dma_start(out=qbf[:], in_=w['qwT'][ct, 0])
            qslabs[(ct, 0)] = qbf
        # key-proj weights next: stageA1(0)'s kproj deps must not queue
        # behind the second qconv slab batch
        keyw_sb = mp.tile([128, 8 * 128], BF16, tag="keyw")
        for kt in range(4):
            nc.sync.dma_start(
                out=keyw_sb[:, kt * 256:(kt + 1) * 256].rearrange("p (m c) -> p m c", m=2),
                in_=w['keywT'][kt].rearrange("m p c -> p m c"))
        # sample-0 A1 loads issued here so kproj(0)/attn(0) deps land early
        kps0 = []
        for ct in range(4):
            kpt = mp.tile([128, 100], BF16, tag="kp", bufs=8)
            nc.sync.dma_start(out=kpt[:], in_=w['kpd'].ap()[0, ct])
            kps0.append(kpt)
        for kt in range(4):
            qbf = mp.tile([128, 1152], BF16, tag="wsbf", bufs=8)
            nc.sync.dma_start(out=qbf[:], in_=w['qwT'][kt, 1])
            qslabs[(kt, 1)] = qbf
        vts0 = []
        for ct in range(4):
            vt = mp.tile([128, 2048], BF16, tag="v", bufs=6)
            nc.sync.dma_start(out=vt[0:100, :], in_=w['vall'].ap()[0, ct])
            vts0.append(vt)

        def load_vec(name, n):
            p = min(n, 128)
            t = mp.tile([128, max(n // 128, 1)], F32, tag=f"vec_{name}")
            nc.sync.dma_start(out=t[0:p, 0:max(n // 128, 1)],
                              in_=w[name].ap().rearrange("(a p) -> p a", p=p))
            return t
        vs = {}
        for nm in ('qs', 'qb'):
            vs[nm] = load_vec(nm, 256)



        ones_bf = mp.tile([128, 1], BF16, tag="ones")
        nc.gpsimd.memset(ones_bf[:], 1.0)

        def x1_s(ct, s):
            base = x1pad[ct][:].rearrange("p (ss a) -> p ss a", ss=B)[:, s]
            return base.rearrange("p (y x) -> p y x", x=22)[:, 1:21, 1:21]

        # ---------- q conv ----------
        q_sb = mp.tile([128, 2 * B * 100], BF16, tag="qsb")
        for mt in range(2):
            ps0 = psC.tile([128, 512], F32, tag="cps", name="cps")
            ps = ps0[:, 0:B * 100]
            first = True
            for kt in range(4):
                base = x1pad[kt][:].rearrange("p (s y x) -> p s y x", s=B, x=22)
                for dy in range(3):
                    for dx in range(3):
                        rhs = base[:, :, dy:dy + 20:2, dx:dx + 20:2]
                        nc.tensor.matmul(
                            ps, qslabs[(kt, mt)][:, (dy * 3 + dx) * 128:(dy * 3 + dx + 1) * 128],
                            rhs, start=first, stop=(kt == 3 and dy == 2 and dx == 2))
                        first = False
            nc.scalar.activation(q_sb[:, mt * B * 100:(mt + 1) * B * 100], ps, AF.Silu,
                                 bias=vs['qb'][:, mt:mt + 1], scale=vs['qs'][:, mt:mt + 1])

        caw1_sb = caw2_sb = c2_sb = None
        sa_sb = {}
        def load_late_weights():
            nonlocal caw1_sb, caw2_sb, c2_sb
            caw1_sb = mp.tile([128, 128], BF16, tag="caw1")
            for kt in range(2):
                nc.sync.dma_start(out=caw1_sb[:, kt * 64:(kt + 1) * 64], in_=w['caw1T'][kt])
            caw2_sb = mp.tile([64, 256], BF16, tag="caw2")
            nc.sync.dma_start(out=caw2_sb[:], in_=w['caw2T'].rearrange("p m c -> p (m c)"))
            for nm, n in [('cab1', 64), ('cab2', 256), ('c3s', 512), ('c3b', 512),
                          ('sa1s', 256), ('sa1b', 256), ('sa2s', 256), ('sa2b', 256),
                          ('c2s', 512), ('c2b', 512)]:
                vs[nm] = load_vec(nm, n)
            c2_sb = mp.tile([128, 6 * 512], BF16, tag="c2w")
            for kt in range(6):
                nc.sync.dma_start(
                    out=c2_sb[:, kt * 512:(kt + 1) * 512].rearrange("p (m c) -> p m c", m=4),
                    in_=w['c2wT'][kt].rearrange("m p c -> p m c"))
            for nm in ('sa1wT', 'sa2wT'):
                for mt in range(2):
                    for kt in range(2):
                        tl = mp.tile([128, 1152], BF16, tag=f"sa_{nm}_{kt}_{mt}")
                        nc.sync.dma_start(out=tl[:], in_=w[nm][kt, mt])
                        sa_sb[(nm, kt, mt)] = tl

        # conv3 weights resident (16 slabs x [128, 1152] bf16); DMAs issued
        # inside the loop at s==0 to keep them off the warmup DMA burst
        c3_sb = mp.tile([128, 16 * 1152], BF16, tag="c3w")

        SCALE = 1.0 / math.sqrt(32)

        # persistent big tensors: aprons zeroed once, interiors rewritten
        attn = []
        for ct in range(4):
            at = mp.tile([128, PADPIX], BF16, tag=f"attn{ct}", name="at", bufs=1)
            attn.append(at)
        xca = []
        xsa1 = []
        a2 = []
        for i in range(2):
            t = mp.tile([128, PADPIX], BF16, tag=f"xca{i}", name="t", bufs=1)
            xca.append(t)
            t2 = mp.tile([128, PADPIX], BF16, tag=f"xsa{i}", name="t2", bufs=1)
            xsa1.append(t2)
            a2t = mp.tile([128, 1600], BF16, tag=f"a2_{i}", name="a2t", bufs=1)
            a2.append(a2t)


        # ---------- stage A1: load host-staged value slabs + key-proj ----------
        def loadA1_kp(s):
            kps = []
            for ct in range(4):
                kpt = mp.tile([128, 100], BF16, tag="kp", bufs=8)
                nc.sync.dma_start(out=kpt[:], in_=w['kpd'].ap()[s, ct])
                kps.append(kpt)
            return kps

        def loadA1_v(s):
            vts = []
            for ct in range(4):
                vt = mp.tile([128, 2048], BF16, tag="v", bufs=6)
                nc.sync.dma_start(out=vt[0:100, :], in_=w['vall'].ap()[s, ct])
                vts.append(vt)
            return vts

        def stageA1(s, pre=None):
            kps, vts = pre if pre is not None else (loadA1_kp(s), loadA1_v(s))
            # key projection
            kk = mp.tile([128, 200], BF16, tag="ksb", bufs=2)
            for mt in range(2):
                psk = psY.tile([128, 512], F32, tag="aps", name="aps")
                for kt in range(4):
                    nc.tensor.matmul(
                        psk[:, 0:100], keyw_sb[:, (kt * 2 + mt) * 128:(kt * 2 + mt + 1) * 128],
                        kps[kt][:], start=(kt == 0), stop=(kt == 3))
                nc.vector.tensor_copy(kk[:, mt * 100:(mt + 1) * 100], psk[:, 0:100])
            return vts, kk

        # ---------- stage A2a: attention scores + exp ----------
        def stageA2a(s, kk):
            exps = []
            for h in range(8):
                emb_ct, emb_off = h // 4, (h % 4) * 32
                pssc = psY.tile([128, 512], F32, tag="aps", name="aps")
                lhs = kk[emb_off:emb_off + 32, emb_ct * 100:(emb_ct + 1) * 100]
                rhs = q_sb[emb_off:emb_off + 32, (emb_ct * B + s) * 100:(emb_ct * B + s + 1) * 100]
                nc.tensor.matmul(pssc[0:100, 0:100], lhs, rhs, tile_position=(emb_off, 0))
                expT = mp.tile([100, 100], BF16, tag="expT", bufs=10)
                nc.scalar.activation(expT[:], pssc[0:100, 0:100], AF.Exp, scale=SCALE)
                exps.append(expT)
            return exps

        # ---------- stage A2b: softmax sums + CA gate + gate apply ----------
        def stageA2b(s, exps):
            recs = []
            for quad in range(2):
                pssum = psY.tile([128, 512], F32, tag="aps", name="aps")
                for hh in range(4):
                    nc.tensor.matmul(pssum[0:1, hh * 100:(hh + 1) * 100],
                                     ones_bf[0:100, 0:1], exps[quad * 4 + hh][:])
                recip4 = mp.tile([1, 400], F32, tag="recip", bufs=2)
                nc.vector.reciprocal(recip4[:], pssum[0:1, 0:400])
                for hh in range(4):
                    rbc = mp.tile([128, 100], F32, tag="rbc", bufs=10)
                    nc.gpsimd.partition_broadcast(rbc[:], recip4[0:1, hh * 100:(hh + 1) * 100])
                    recs.append((exps[quad * 4 + hh], rbc))
            # CA gate at 20x20: the gate logits are 2x2-block constant under
            # the nearest-neighbor upsample, so compute the MLP on the 20x20
            # grid (4x fewer MACs) and broadcast in the apply multiply.
            hps = psC.tile([64, 512], F32, tag="cps", name="cps")
            for i in range(2):
                nc.tensor.matmul(hps[:, 0:400], caw1_sb[:, i * 64:(i + 1) * 64],
                                 x1_s(2 + i, s), start=(i == 0), stop=(i == 1))
            hsb = mp.tile([64, 400], BF16, tag="hsb", bufs=2)
            nc.scalar.activation(hsb[:], hps[:, 0:400], AF.Relu, bias=vs['cab1'][0:64, 0:1])
            gts = []
            for mt in range(2):
                gps = psC.tile([128, 512], F32, tag="cps", name="cps")
                nc.tensor.matmul(gps[:, 0:400], caw2_sb[0:64, mt * 128:(mt + 1) * 128], hsb[:])
                gt = mp.tile([128, 400], BF16, tag="gate", bufs=4)
                nc.scalar.activation(gt[:], gps[:, 0:400], AF.Sigmoid,
                                     bias=vs['cab2'][:, mt:mt + 1])
                gts.append(gt)
            for i in range(2):
                dst = xca[i][:].rearrange("p (y x) -> p y x", x=WP)[:, 1:41, 1:41]
                dst5 = dst.rearrange("p (yy r) (xx px) -> p yy r xx px", r=2, px=2)
                a14 = x1_s(2 + i, s).unsqueeze(3).broadcast_to([128, 20, 20, 2])
                g4 = gts[i][:].rearrange("p (y x) -> p y x", x=20)
                g4 = g4.unsqueeze(3).broadcast_to([128, 20, 20, 2])
                for r in range(2):
                    nc.vector.tensor_tensor(dst5[:, :, r], a14, g4, AluOpType.mult)
            return recs

        def conv3x3_mt(src_tiles, slab_fn, mt, kt_n):
            pss = []
            for chunk in range(NCHUNK):
                ps0 = psC.tile([128, 512], F32, tag="cps", name="cps")
                y0 = chunk * CHUNK_ROWS
                first = True
                for kt in range(kt_n):
                    base = src_tiles[kt][:].rearrange("p (y x) -> p y x", x=WP)
                    for dy in range(3):
                        for dx in range(3):
                            rhs = base[:, y0 + dy:y0 + dy + CHUNK_ROWS, dx:dx + 40]
                            lhsT = slab_fn(kt, mt, dy * 3 + dx)
                            nc.tensor.matmul(ps0[:, 0:CHUNK_N], lhsT, rhs, start=first,
                                             stop=(kt == kt_n - 1 and dy == 2 and dx == 2))
                            first = False
                pss.append(ps0)
            return pss

        # ---------- main loop ----------
        vts, kk0 = stageA1(0, pre=(kps0, vts0))
        exps0 = stageA2a(0, kk0)
        load_late_weights()
        for t in attn + xca + xsa1:
            _apron_memset(nc, t)
        recs = stageA2b(0, exps0)
        for s in range(B):
            # ---- SA conv1 ----
            for mt in range(2):
                pss = conv3x3_mt(
                    xca, lambda kt, m, t: sa_sb[('sa1wT', kt, m)][:, t * 128:(t + 1) * 128],
                    mt, 2)
                for chunk in range(NCHUNK):
                    ps = pss[chunk][:, 0:CHUNK_N]
                    y0 = chunk * CHUNK_ROWS
                    dst = xsa1[mt][:].rearrange("p (y x) -> p y x", x=WP)[:, 1 + y0:11 + y0, 1:41]
                    nc.scalar.activation(dst, ps.rearrange("p (a b) -> p a b", b=40), AF.Silu,
                                         bias=vs['sa1b'][:, mt:mt + 1], scale=vs['sa1s'][:, mt:mt + 1])

            # ---- attention apply: 4 patch-positions per psum bank ----
            for h in range(8):
                expT, rbc = recs[h]
                o = (h % 2) * 64
                for py in range(4):
                    psy = psY.tile([128, 512], F32, tag="aps", name="aps")
                    for px in range(4):
                        pp = py * 4 + px
                        lhsT = vts[h // 2][0:100, pp * 128 + o: pp * 128 + o + 64]
                        nc.tensor.matmul(psy[o:o + 64, px * 100:px * 100 + 100], lhsT, expT[:],
                                         start=(px == 0), stop=(px == 3))
                    dstg = attn[h // 2][o:o + 64, :].rearrange("p (y x) -> p y x", x=WP)
                    dstg = dstg[:, 1 + py:38 + py:4, 1:41]
                    dstg = dstg.rearrange("p a (pwx px) -> p a pwx px", px=4)
                    in0 = psy[o:o + 64, 0:400].rearrange("p (px phy pwx) -> p phy pwx px",
                                                         px=4, phy=10)
                    in1 = rbc[o:o + 64, :].rearrange("p (a b) -> p a b", b=10)
                    in1 = in1.unsqueeze(3).broadcast_to([64, 10, 10, 4])
                    nc.vector.scalar_tensor_tensor(dstg, in0, 0.0, in1,
                                                   AluOpType.bypass, AluOpType.mult)

            # ---- prefetch next sample's A1 stage + scores/exp ----
            if s + 1 < B:
                next_vts, next_kk = stageA1(s + 1)
                next_exps = stageA2a(s + 1, next_kk)

            # ---- SA conv2 + residual ----
            for mt in range(2):
                pss = conv3x3_mt(
                    xsa1, lambda kt, m, t: sa_sb[('sa2wT', kt, m)][:, t * 128:(t + 1) * 128],
                    mt, 2)
                for chunk in range(NCHUNK):
                    ps = pss[chunk][:, 0:CHUNK_N]
                    y0 = chunk * CHUNK_ROWS
                    tsilu = mp.tile([128, CHUNK_N], F32, tag="silu", bufs=4)
                    nc.scalar.activation(tsilu[:], ps, AF.Silu,
                                         bias=vs['sa2b'][:, mt:mt + 1], scale=vs['sa2s'][:, mt:mt + 1])
                    xc = xca[mt][:].rearrange("p (y x) -> p y x", x=WP)[:, 1 + y0:11 + y0, 1:41]
                    nc.vector.tensor_tensor(a2[mt][:, y0 * 40:(y0 + 10) * 40],
                                            tsilu[:].rearrange("p (a b) -> p a b", b=40),
                                            xc, AluOpType.add)

            # ---- conv3 (resident bf16 slabs) + residual -> x2_out ----
            if s == 0:
                for mt in range(4):
                    for kt in range(4):
                        off = (mt * 4 + kt) * 1152
                        nc.sync.dma_start(out=c3_sb[:, off:off + 1152], in_=w['c3wT'][kt, mt])
            for mt in range(2):
                pss = conv3x3_mt(
                    attn,
                    lambda kt, m, t: c3_sb[:, (m * 4 + kt) * 1152 + t * 128:
                                           (m * 4 + kt) * 1152 + (t + 1) * 128],
                    mt, 4)
                for chunk in range(NCHUNK):
                    ps = pss[chunk][:, 0:CHUNK_N]
                    y0 = chunk * CHUNK_ROWS
                    tsilu = mp.tile([128, CHUNK_N], F32, tag="silu", bufs=4)
                    nc.scalar.activation(tsilu[:], ps, AF.Silu,
                                         bias=vs['c3b'][:, mt:mt + 1], scale=vs['c3s'][:, mt:mt + 1])
                    osb = mp.tile([128, CHUNK_N], F32, tag="osb", bufs=4)
                    at2 = attn[mt][:].rearrange("p (y x) -> p y x", x=WP)[:, 1 + y0:11 + y0, 1:41]
                    nc.vector.tensor_tensor(osb[:].rearrange("p (a b) -> p a b", b=40),
                                            tsilu[:].rearrange("p (a b) -> p a b", b=40),
                                            at2, AluOpType.add)
                    nc.sync.dma_start(
                        out=out.ap()[s, 512 + mt * 128:512 + (mt + 1) * 128]
                            .rearrange("p y x -> p (y x)")[:, y0 * 40:(y0 + 10) * 40],
                        in_=osb[:])

            # ---- prefetch next sample's softmax sums + CA gate mid-conv3 ----
            if s + 1 < B:
                next_recs = stageA2b(s + 1, next_exps)

            for mt in range(2, 4):
                pss = conv3x3_mt(
                    attn,
                    lambda kt, m, t: c3_sb[:, (m * 4 + kt) * 1152 + t * 128:
                                           (m * 4 + kt) * 1152 + (t + 1) * 128],
                    mt, 4)
                for chunk in range(NCHUNK):
                    ps = pss[chunk][:, 0:CHUNK_N]
                    y0 = chunk * CHUNK_ROWS
                    tsilu = mp.tile([128, CHUNK_N], F32, tag="silu", bufs=4)
                    nc.scalar.activation(tsilu[:], ps, AF.Silu,
                                         bias=vs['c3b'][:, mt:mt + 1], scale=vs['c3s'][:, mt:mt + 1])
                    osb = mp.tile([128, CHUNK_N], F32, tag="osb", bufs=4)
                    at2 = attn[mt][:].rearrange("p (y x) -> p y x", x=WP)[:, 1 + y0:11 + y0, 1:41]
                    nc.vector.tensor_tensor(osb[:].rearrange("p (a b) -> p a b", b=40),
                                            tsilu[:].rearrange("p (a b) -> p a b", b=40),
                                            at2, AluOpType.add)
                    nc.sync.dma_start(
                        out=out.ap()[s, 512 + mt * 128:512 + (mt + 1) * 128]
                            .rearrange("p y x -> p (y x)")[:, y0 * 40:(y0 + 10) * 40],
                        in_=osb[:])

            # ---- conv2 (1x1) -> x1_out ----
            # x1pad channels are 2x2-block constant: contract them once per mt
            # on the 20x20 grid, then add (broadcast) to the 40x40 a2 part.
            for mt in range(4):
                psx = psY.tile([128, 512], F32, tag="aps", name="aps")
                for kt in range(4):
                    nc.tensor.matmul(psx[:, 0:400],
                                     c2_sb[:, (kt * 4 + mt) * 128:(kt * 4 + mt + 1) * 128],
                                     x1_s(kt, s), start=(kt == 0), stop=(kt == 3))
                x1c = mp.tile([128, 400], F32, tag="x1c", bufs=2)
                nc.vector.tensor_copy(x1c[:], psx[:, 0:400])
                for chunk in range(NCHUNK):
                    ps0 = psC.tile([128, 512], F32, tag="cps", name="cps")
                    ps = ps0[:, 0:CHUNK_N]
                    y0 = chunk * CHUNK_ROWS
                    for i in range(2):
                        rhs = a2[i][:, y0 * 40:(y0 + 10) * 40]
                        nc.tensor.matmul(ps, c2_sb[:, ((4 + i) * 4 + mt) * 128:((4 + i) * 4 + mt + 1) * 128],
                                         rhs, start=(i == 0), stop=(i == 1))
                    tsum = mp.tile([128, CHUNK_N], F32, tag="silu", bufs=4)
                    t5 = tsum[:].rearrange("p (yy r xx px) -> p yy r xx px", r=2, xx=20, px=2)
                    p5 = ps.rearrange("p (yy r xx px) -> p yy r xx px", r=2, xx=20, px=2)
                    xc4 = x1c[:, chunk * 100:(chunk + 1) * 100].rearrange("p (y x) -> p y x", x=20)
                    xc4 = xc4.unsqueeze(3).broadcast_to([128, 5, 20, 2])
                    for r in range(2):
                        nc.vector.tensor_tensor(t5[:, :, r], p5[:, :, r], xc4, AluOpType.add)
                    osb = mp.tile([128, CHUNK_N], F32, tag="osb", bufs=4)
                    nc.scalar.activation(osb[:], tsum[:], AF.Silu,
                                         bias=vs['c2b'][:, mt:mt + 1], scale=vs['c2s'][:, mt:mt + 1])
                    nc.sync.dma_start(
                        out=out.ap()[s, mt * 128:(mt + 1) * 128]
                            .rearrange("p y x -> p (y x)")[:, y0 * 40:(y0 + 10) * 40],
                        in_=osb[:])

            if s + 1 < B:
                vts, recs = next_vts, next_recs
